# revision 47
# baseline (speedup 1.0000x reference)
"""AttentionPool Trainium2 Bass kernel (w2-mass-aware precision, fp8 DR).

Reference computation (per batch b):
    h      = tanh(x @ W1 + b1)          # [N, H*F]
    scores = h @ W2 + b2                # [N, H]   (b2 cancels under softmax)
    scores = where(mask, scores, -1e9)
    w      = softmax(scores, axis=N)
    pooled = w.T @ x                    # [H, D]
    y      = concat_h(pooled) @ Wout + bout

Strategy (vs the 151us 8-resid-pass baseline):
 1. Host-side valid-token compaction (~50% of tokens masked) and batch
    count-sorting into (core, slot), as before.
 2. Each head's F=512 hidden dims are PERMUTED so large-|w2| dims come
    first.  Per head, chunk 0 (128 dims) carries ~72% of sum(w2^2),
    chunk 1 ~20%, chunks 2-3 ~7%.  Score noise scales with the w2^2
    mass of the chunk it enters through, so precision is allocated by
    chunk rank:
      - main x8@w18 fp8 DoubleRow pass: all 16 mc chunks (irreducible)
      - residual passes ONLY on the 4 rank-0 chunks: x-residual on
        d>=256 (xlo@w18) + W-residual on d>=512 (x8@w1lo)
      - score dot h@W2 entirely in fp8 DoubleRow (tanh emits fp8
        directly); top pairs (ranks 0,1) get a w2-residual second pass.
    48 PE-cycles/token vs baseline's 80.  numpy-sim absmax 1.59e-2,
    HW-measured 1.57e-2 (gate 2e-2; baseline measured 1.52e-2).
 3. b1 is applied exactly as a per-partition fp32 bias AP in the tanh
    activation (tanh runs per-mc chunk), replacing the ones-row trick.
    b2 cancels under softmax.  w2 is pre-scaled by 8 (power of two,
    exact) so fp8 quantization stays out of subnormals; the exp
    activation un-scales with scale=1/8.
 4. NO pad mask: pad token columns are all-zero in xt8/xlo, so their
    h8 is exactly q8(tanh(b1)) and their pooling contribution is 0
    (xc rows are zero).  They only inflate the softmax denominator by
    (npad-cnt)*e^(s_pad), which the host computes exactly and the
    device subtracts (zcorrn input).  exp reads the score PSUM
    directly per block (accum per block), killing the DVE mask-add
    and the m16 tensor.
 5. Token blocks of TB=1024 (PSUM [128,1024] tiles, ring of 3).  The
    score dot is software-pipelined two pairs behind the h chains so
    the PE never waits on the tanh of the pair it just produced.
 6. The whole pooling path (e weights, x, Wout, pooled) runs fp16.

Layouts (d = dc*256 + i*128 + p for DoubleRow pairs; f = mc*128 + p
with mc = 4*head + rank after the per-head w2-sort):
  xt8  [BL, P, 4, 2, N_pad] fp8   x compacted, transposed, e4m3
  xlo  [BL, P, 4, 2, N_pad] fp8   q8(x - x8), all d
  xc   [BL, N_pad, D]      fp16   natural x for pooling
  w18  [P, 4, 2, HF] fp8          q8(8*W1f)  (W1f column-permuted)
  w1lo [P, 4, 2, 4*128] fp8       q8(8*W1f - w18), rank-0 cols, head-major
  w28  [P, 8, 2, 16] fp8          q8(8*w2) one-hot-by-head, DR pairs
                                  (head axis padded 4->16: DR LdWeights
                                  needs pair-axis byte-step % 16 == 0)
  w28lo[P, 4, 2, 16] fp8          q8(8*w2 - w28) for top pairs
  b1T  [P, 16] fp32               b1 per (p, mc), tanh bias APs
  zcorrn [H, BL] fp32             -(npad-cnt)*e^(s_pad) denominator fix
  wout [P, 32, D] fp16, boutT [P, 8, BL] fp32, y [P, 8, BL] fp32
"""

import numpy as np
import ml_dtypes

import concourse.bass as bass
import concourse.mybir as mybir
import concourse.tile as tile
from concourse import bacc
from concourse.bass import ts
from concourse.bass_utils import run_bass_kernel_spmd
from concourse.masks import make_identity

FP32 = mybir.dt.float32
F8 = mybir.dt.float8e4
FP16 = mybir.dt.float16
AFT = mybir.ActivationFunctionType
DR = mybir.MatmulPerfMode.DoubleRow

P = 128
TB = 1024


def _blocks(n_pad, tb=TB):
    """Split a slot into token blocks.  Oversize slots split into two
    BALANCED blocks (not 1024+tail): the Act engine's fixed per-tanh
    overhead makes tiny tail blocks Act-bound, stalling the next slot
    on the PSUM ring."""
    if n_pad <= tb:
        return [(0, n_pad)]
    assert n_pad <= 2 * tb
    b0 = (-(-n_pad // 2) + 15) // 16 * 16
    return [(0, b0), (b0, n_pad - b0)]


class Cfg:
    def __init__(self, BL=4, N=2048, D=1024, H=4, F=512, N_pad=1152,
                 slot_npads=None):
        self.BL, self.N, self.D, self.H, self.F = BL, N, D, H, F
        self.HF = H * F
        self.N_pad = N_pad
        self.KDR = D // 256          # DoubleRow d-chunks (256 each)
        self.MC = self.HF // P       # h col chunks (16)
        self.PC = self.MC // 2       # score-dot pairs (8)
        self.KD = D // P             # 128-chunks of D
        self.KOUT = (H * D) // P     # contraction chunks of the out proj
        self.slot_npads = list(slot_npads) if slot_npads else [N_pad] * BL
        assert len(self.slot_npads) == BL
        assert max(self.slot_npads) <= N_pad
        self.slot_blocks = [_blocks(np_) for np_ in self.slot_npads]
        self.slot_chunks = []
        for np_ in self.slot_npads:
            ch, n0 = [], 0
            while n0 < np_:
                s = min(P, np_ - n0)
                ch.append((n0, s))
                n0 += s
            self.slot_chunks.append(ch)
        self.CHM = max(12, max(len(ch) for ch in self.slot_chunks))


def build_kernel(nc: bass.Bass, cfg: Cfg, reps: int = 1):
    c = cfg
    xt8_d = nc.dram_tensor("xt8", [c.BL, P, c.KDR, 2, c.N_pad], F8,
                           kind="ExternalInput").ap()
    xlo_d = nc.dram_tensor("xlo", [c.BL, P, c.KDR, 2, c.N_pad], F8,
                           kind="ExternalInput").ap()
    xc_d = nc.dram_tensor("xc", [c.BL, c.N_pad, c.D], FP16,
                          kind="ExternalInput").ap()
    w18_d = nc.dram_tensor("w18", [P, c.KDR, 2, c.HF], F8,
                           kind="ExternalInput").ap()
    w1lo_d = nc.dram_tensor("w1lo", [P, c.KDR, 2, c.H * P], F8,
                            kind="ExternalInput").ap()
    w28_d = nc.dram_tensor("w28", [P, c.PC, 2, 16], F8,
                           kind="ExternalInput").ap()
    w28lo_d = nc.dram_tensor("w28lo", [P, c.PC // 2, 2, 16], F8,
                             kind="ExternalInput").ap()
    b1T_d = nc.dram_tensor("b1T", [P, c.MC], FP32,
                           kind="ExternalInput").ap()
    zcn_d = nc.dram_tensor("zcorrn", [c.H, c.BL], FP32,
                           kind="ExternalInput").ap()
    wout_d = nc.dram_tensor("wout", [P, c.KOUT, c.D], FP16,
                            kind="ExternalInput").ap()
    boutT_d = nc.dram_tensor("boutT", [P, c.KD, c.BL], FP32,
                             kind="ExternalInput").ap()
    y_d = nc.dram_tensor("y", [P, c.KD, c.BL], FP32,
                         kind="ExternalOutput").ap()

    with tile.TileContext(nc) as tc:
        with (
            tc.tile_pool(name="const", bufs=1) as const,
            tc.tile_pool(name="xt8p", bufs=3) as xt8_pool,
            tc.tile_pool(name="xlop", bufs=3) as xlo_pool,
            tc.tile_pool(name="h8p", bufs=12) as h8_pool,
            tc.tile_pool(name="xnp", bufs=20) as xn_pool,
            tc.tile_pool(name="ep", bufs=2) as e_pool,
            tc.tile_pool(name="etp", bufs=2) as eT_pool,
            tc.tile_pool(name="smallp", bufs=4) as small_pool,
            tc.tile_pool(name="hps", bufs=3, space="PSUM") as hps_pool,
            tc.tile_pool(name="scps", bufs=1, space="PSUM") as scps_pool,
        ):
            # ---- constants.  scalar/Act queue carries NO DMAs: the
            # issuing engine is blocked for the whole transfer in this
            # model, and Act is the co-bottleneck (tanh).  w18 streams in
            # pair-column slices ordered by first use (odd pairs first,
            # matching block 0's pc_order); b1T leads for the first tanh.
            # x-data rides the SP queue. ----
            w18_sb = const.tile([P, c.KDR, 2, c.HF], F8)
            w1lo_sb = const.tile([P, c.KDR, 2, c.H * P], F8)
            w28_sb = const.tile([P, c.PC, 2, 16], F8)
            w28lo_sb = const.tile([P, c.PC // 2, 2, 16], F8)
            b1T_sb = const.tile([P, c.MC], FP32)
            zcn_sb = const.tile([c.H, c.BL], FP32)
            for pc in (1, 3):
                cs = ts(pc, 256)
                nc.gpsimd.dma_start(w18_sb[:, :, :, cs], w18_d[:, :, :, cs])
            nc.gpsimd.dma_start(b1T_sb[:], b1T_d)
            for pc in (5, 7):
                cs = ts(pc, 256)
                nc.gpsimd.dma_start(w18_sb[:, :, :, cs], w18_d[:, :, :, cs])
            nc.gpsimd.dma_start(w1lo_sb[:], w1lo_d)
            nc.gpsimd.dma_start(w18_sb[:, :, :, ts(0, 256)],
                                w18_d[:, :, :, ts(0, 256)])
            nc.gpsimd.dma_start(w28_sb[:], w28_d)
            nc.gpsimd.dma_start(w28lo_sb[:], w28lo_d)
            for pc in (2, 4, 6):
                cs = ts(pc, 256)
                nc.gpsimd.dma_start(w18_sb[:, :, :, cs], w18_d[:, :, :, cs])
            nc.gpsimd.dma_start(zcn_sb[:], zcn_d)
            # wout, boutT are DMA'd from inside the first block's emission
            # so they queue behind the startup-critical transfers
            boutT_sb = const.tile([P, c.KD, c.BL], FP32)
            wout_sb = const.tile([P, c.KOUT, c.D], FP16)
            idH = const.tile([c.H, c.H], FP16)
            make_identity(nc, idH[:])
            idHf = const.tile([c.H, c.H], FP32)
            make_identity(nc, idHf[:])
            ones4 = const.tile([c.H, P], FP32)
            nc.vector.memset(ones4[:], 1.0)
            # warm the activation table (Tanh/Exp share one set) during
            # the initial DMA wait
            warm = const.tile([1, 1], FP32)
            nc.scalar.activation(warm[:], idH[:1, :1], AFT.Tanh)
            poolT_sb = const.tile([P, c.KD, c.H, c.BL], FP16)
            y_sbT = const.tile([P, c.KD, c.BL], FP32)

            def emit_xn_dmas(b):
                tiles = []
                for n0, cs in c.slot_chunks[b]:
                    xn = xn_pool.tile([P, c.D], FP16, tag="xn")
                    nc.gpsimd.dma_start(xn[:cs], xc_d[b, n0:n0 + cs, :])
                    tiles.append(xn)
                return tiles

            def emit_scores(b, rep):
                """DMA + h matmuls + tanh(fp8 out) + fp8-DR score dot +
                per-block exp straight off the score PSUM."""
                # +128 columns so a partial tail chunk transposes as a
                # full 128-wide chunk (zeroed here, off the critical path)
                e_bf = e_pool.tile([c.H, c.N_pad + 128], FP16, tag="e")
                zsb = small_pool.tile([c.H, len(c.slot_blocks[b])],
                                      FP32, tag="zsb")
                nbv = c.slot_npads[b]
                padv = (-nbv) % P
                if padv:
                    nc.vector.memset(e_bf[:, nbv:nbv + padv], 0.0)
                first = b == 0 and rep == 0
                for blk, (n0, tb) in enumerate(c.slot_blocks[b]):
                    xt8 = xt8_pool.tile([P, c.KDR, 2, TB], F8, tag="xt8")
                    xlo = xlo_pool.tile([P, c.KDR, 2, TB], F8, tag="xlo")
                    if first and blk == 0:
                        # per-dc split so the first chains start as soon
                        # as their own chunk lands
                        for dc in range(c.KDR):
                            nc.sync.dma_start(
                                xt8[:, dc, :, :tb],
                                xt8_d[b, :, dc, :, n0:n0 + tb],
                            )
                        for dc in range(c.KDR):
                            nc.sync.dma_start(
                                xlo[:, dc, :, :tb],
                                xlo_d[b, :, dc, :, n0:n0 + tb],
                            )
                        nc.gpsimd.dma_start(wout_sb[:], wout_d)
                        nc.gpsimd.dma_start(boutT_sb[:], boutT_d)
                    else:
                        nc.sync.dma_start(
                            xt8[:, :, :, :tb], xt8_d[b, :, :, :, n0:n0 + tb]
                        )
                        nc.sync.dma_start(
                            xlo[:, :, :, :tb], xlo_d[b, :, :, :, n0:n0 + tb]
                        )
                    # first block: residual-free odd pairs first so the
                    # PE isn't gated on the xlo DMA splits at startup
                    if first and blk == 0:
                        pc_order = [1, 3, 5, 7, 0, 2, 4, 6]
                    else:
                        pc_order = list(range(c.PC))
                    h8s = [None] * c.PC
                    hv = [(s0, min(512, tb - s0)) for s0 in range(0, tb, 512)]
                    scp = scps_pool.tile([16, TB], FP32, tag="scps")
                    # each 512-col PSUM half is its own accumulation
                    # group: first/last instr per half carry start/stop
                    _per_half = c.PC + c.PC // 2
                    n_sc = [0] * len(hv)

                    def emit_score(pc):
                        for hi, (s0, sw) in enumerate(hv):
                            sl = slice(s0, s0 + sw)
                            nc.tensor.matmul(
                                scp[:, sl], w28_sb[:, pc],
                                h8s[pc][:, :, sl],
                                start=(n_sc[hi] == 0),
                                stop=(n_sc[hi] == _per_half - 1),
                                perf_mode=DR,
                            )
                            n_sc[hi] += 1
                            if pc % 2 == 0:
                                nc.tensor.matmul(
                                    scp[:, sl], w28lo_sb[:, pc // 2],
                                    h8s[pc][:, :, sl],
                                    start=False,
                                    stop=(n_sc[hi] == _per_half - 1),
                                    perf_mode=DR,
                                )
                                n_sc[hi] += 1
                    pend = []
                    for pc in pc_order:
                        h8 = h8_pool.tile([P, 2, TB], F8, tag="h8")
                        top = pc % 2 == 0   # ranks {0,1} pair of head pc//2
                        hd = pc // 2
                        for j in range(2):
                            mc = 2 * pc + j
                            r0 = top and j == 0  # rank-0 chunk
                            ms = ts(mc, P)
                            hp = hps_pool.tile([P, TB], FP32, tag="h")
                            for s0, sw in hv:
                                sl = slice(s0, s0 + sw)
                                for dc in range(c.KDR):
                                    nc.tensor.matmul(
                                        hp[:, sl],
                                        w18_sb[:, dc, :, ms],
                                        xt8[:, dc, :, sl],
                                        start=(dc == 0),
                                        stop=(dc == c.KDR - 1 and not r0),
                                        perf_mode=DR,
                                    )
                                if r0:
                                    # x-resid on d>=256 + W-resid d>=512
                                    # (the d<256 quarter is dropped: its
                                    # noise share is small vs 2 cyc/tok)
                                    for dc in (1, 2, 3):
                                        nc.tensor.matmul(
                                            hp[:, sl],
                                            w18_sb[:, dc, :, ms],
                                            xlo[:, dc, :, sl],
                                            start=False, stop=False,
                                            perf_mode=DR,
                                        )
                                    for dc in (2, 3):
                                        nc.tensor.matmul(
                                            hp[:, sl],
                                            w1lo_sb[:, dc, :, ts(hd, P)],
                                            xt8[:, dc, :, sl],
                                            start=False, stop=(dc == 3),
                                            perf_mode=DR,
                                        )
                            nc.scalar.activation(
                                h8[:, j, :tb], hp[:, :tb], AFT.Tanh,
                                bias=b1T_sb[:, mc:mc + 1], scale=0.125,
                            )
                        h8s[pc] = h8
                        # score dot runs two pairs behind the chains so
                        # the PE never waits on the tanh it just fed
                        pend.append(pc)
                        lag = 8 if (first and blk > 0) else 4
                        if len(pend) > lag:
                            emit_score(pend.pop(0))
                    for pc in pend:
                        emit_score(pc)
                    # per-block exp straight off the score PSUM (scores
                    # are at 8x scale; pad tokens included, fixed via
                    # zcorrn in the softmax denominator)
                    nc.scalar.activation(
                        e_bf[:, n0:n0 + tb], scp[:c.H, :tb],
                        AFT.Exp, bias=0.0, scale=0.125,
                        accum_out=zsb[:, blk:blk + 1],
                    )
                xn_tiles = emit_xn_dmas(b)
                return (e_bf, zsb), xn_tiles

            def emit_tail(b, e_zsb, xn_tiles):
                """softmax + pooling for batch b (runs under b+1's scores).

                Pooling contracts the RAW (unnormalized) e values; the
                1/Z per-head scale is applied to the tiny pooled [D, H]
                matrix at the end, so the transpose/pool chain never
                waits on the denominator.  Z reaches all 128 partitions
                via ones.T @ (idH * zs).
                """
                e_bf, zsb = e_zsb
                zs = small_pool.tile([c.H, 1], FP32, tag="zs")
                # denominator: sum block accums, subtract the host-computed
                # pad contribution (zcorrn is negated on the host)
                nc.vector.tensor_add(
                    zs[:], zsb[:, 0:1], zcn_sb[:, b:b + 1]
                )
                for blk in range(1, len(c.slot_blocks[b])):
                    nc.vector.tensor_add(zs[:], zs[:], zsb[:, blk:blk + 1])
                zdiag = small_pool.tile([c.H, c.H], FP32, tag="zdiag")
                nc.vector.tensor_scalar_mul(zdiag[:], idHf[:], zs[:])
                chunks = c.slot_chunks[b]
                ncb = len(chunks)
                tpt = hps_pool.tile([P, c.CHM, c.H], FP16, tag="h")
                for cn, (n0, cs) in enumerate(chunks):
                    nc.tensor.matmul(
                        tpt[:, cn, :], e_bf[:, n0:n0 + P], idH[:],
                        is_transpose=True,
                        start=(cn == 0), stop=(cn == ncb - 1),
                    )
                eTt = eT_pool.tile([P, c.CHM, c.H], FP16, tag="eT")
                nc.vector.tensor_copy(eTt[:, :ncb], tpt[:, :ncb])
                zbc = hps_pool.tile([P, c.H], FP32, tag="h")
                nc.tensor.matmul(zbc[:], ones4[:], zdiag[:],
                                 start=True, stop=True)
                rzbc = small_pool.tile([P, c.H], FP32, tag="rzbc")
                nc.vector.reciprocal(rzbc[:], zbc[:])
                plt = hps_pool.tile([P, c.KD, c.H], FP32, tag="h")
                for dc in range(c.KD):
                    for cn, (n0, cs) in enumerate(chunks):
                        nc.tensor.matmul(
                            plt[:, dc, :], xn_tiles[cn][:cs, ts(dc, P)],
                            eTt[:cs, cn, :],
                            start=(cn == 0), stop=(cn == ncb - 1),
                        )
                for hd in range(c.H):
                    nc.vector.tensor_scalar_mul(
                        poolT_sb[:, :, hd, b], plt[:, :, hd],
                        rzbc[:, hd:hd + 1],
                    )
                # out projection for this batch's column while later
                # batches still stream
                ytp = hps_pool.tile([P, c.KD, 1], FP32, tag="h")
                for dblk in range(c.KD):
                    for kc in range(c.KOUT):
                        hd, dc = kc // c.KD, kc % c.KD
                        nc.tensor.matmul(
                            ytp[:, dblk, :],
                            wout_sb[:, kc, ts(dblk, P)],
                            poolT_sb[:, dc, hd, b:b + 1],
                            start=(kc == 0), stop=(kc == c.KOUT - 1),
                        )
                nc.vector.tensor_add(
                    y_sbT[:, :, b:b + 1], ytp[:], boutT_sb[:, :, b:b + 1]
                )

            for rep in range(reps):
                prev = None
                for b in range(c.BL):
                    e_zsb, xn_tiles = emit_scores(b, rep)
                    if prev is not None:
                        emit_tail(prev[0], prev[1], prev[2])
                    prev = (b, e_zsb, xn_tiles)
                emit_tail(prev[0], prev[1], prev[2])
                nc.sync.dma_start(y_d[:], y_sbT[:])
    return nc


def plan_slots(valid_mask, n_cores, BL):
    """Count-sort batches into (core, slot) so each SPMD batch-slot has a
    tight shared token bound."""
    counts = np.asarray(valid_mask).sum(1)
    order = np.argsort(counts, kind="stable")[::-1]
    slot_npads = []
    for bl in range(BL):
        grp = order[bl * n_cores:(bl + 1) * n_cores]
        mx = int(counts[grp].max())
        slot_npads.append(max(256, int(np.ceil(mx / 16) * 16)))
    return order, slot_npads


def make_in_maps(x, valid_mask, W1, b1, W2, b2, Wout, bout, n_cores, cfg):
    """Host-side prep: w2-sort heads' dims, compact valid tokens, fp8
    layouts, shard over batch."""
    c = cfg
    f8 = ml_dtypes.float8_e4m3
    f16 = np.float16
    B, N, D = x.shape
    H, _, F = W1.shape
    HF = H * F

    def q8(a):
        return np.asarray(a, np.float32).astype(f8)

    # per-head permutation: large |w2| dims first
    perm = [np.argsort(-np.abs(np.asarray(W2[h], np.float32)),
                       kind="stable") for h in range(H)]
    W1p = np.stack([np.asarray(W1[h], np.float32)[:, perm[h]]
                    for h in range(H)])          # [H, D, F]
    b1p = np.stack([np.asarray(b1[h], np.float32)[perm[h]]
                    for h in range(H)])          # [H, F]
    w2p = np.stack([np.asarray(W2[h], np.float32)[perm[h]]
                    for h in range(H)])          # [H, F]

    W1f = W1p.transpose(1, 0, 2).reshape(D, HF)  # [D, HF] head-major cols
    w18 = q8(8.0 * W1f)
    w1r = 8.0 * W1f - w18.astype(np.float32)

    def dr_pack_w(wmat):
        # [D, M] -> [P, D//256, 2, M] with d = dc*256 + i*128 + p
        Dw, M = wmat.shape
        return np.ascontiguousarray(
            wmat.reshape(Dw // 256, 2, P, M).transpose(2, 0, 1, 3)
        )

    w18_l = dr_pack_w(w18)
    # rank-0 columns of each head, head-major compact [D, H*P]
    r0cols = np.concatenate(
        [np.arange(4 * h * P, (4 * h + 1) * P) for h in range(H)]
    )
    w1lo_l = dr_pack_w(q8(w1r[:, r0cols]))

    # score-dot stationaries: w28[p, pc, j, hd] = q8(8*w2[f]) one-hot by
    # head, f = (2*pc+j)*128 + p, head = (2*pc+j)//4.  Head axis padded
    # 4->16 for the DR LdWeights pair-step%16 rule.
    w2f8 = q8(8.0 * w2p.reshape(HF))
    w2flo = q8(8.0 * w2p.reshape(HF) - w2f8.astype(np.float32))
    w28_l = np.zeros((P, c.PC, 2, 16), f8)
    w28lo_l = np.zeros((P, c.PC // 2, 2, 16), f8)
    for pc in range(c.PC):
        for j in range(2):
            mc = 2 * pc + j
            fidx = mc * P + np.arange(P)
            w28_l[np.arange(P), pc, j, mc // 4] = w2f8[fidx]
            if pc % 2 == 0:
                w28lo_l[np.arange(P), pc // 2, j, mc // 4] = w2flo[fidx]
    w28_l = np.ascontiguousarray(w28_l)
    w28lo_l = np.ascontiguousarray(w28lo_l)

    b1T_l = np.ascontiguousarray(
        b1p.reshape(HF).reshape(c.MC, P).transpose(1, 0).astype(np.float32)
    )

    # pad-token score per head at the device's exact precision:
    # h_pad = e4m3(tanh(b1)), s_pad8 = sum_f w28[f]*h_pad[f] (+ w28lo on
    # top pairs); e_pad = exp(s_pad8/8)
    hpadq = np.asarray(np.tanh(b1p.reshape(HF)), np.float32).astype(f8)
    hpadf = hpadq.astype(np.float32)
    w28f = w2f8.astype(np.float32)
    w28lof = w2flo.astype(np.float32)
    s_pad8 = np.zeros(H, np.float64)
    for mc in range(c.MC):
        fidx = mc * P + np.arange(P)
        hd = mc // 4
        s_pad8[hd] += (w28f[fidx] * hpadf[fidx]).sum()
        if (mc // 2) % 2 == 0:   # top pair -> w2lo residual applies
            s_pad8[hd] += (w28lof[fidx] * hpadf[fidx]).sum()
    e_pad = np.exp(s_pad8 / 8.0)                 # [H]

    wout_l = np.ascontiguousarray(
        np.asarray(Wout, np.float32).reshape(c.KOUT, P, c.D)
        .transpose(1, 0, 2).astype(f16)
    )
    boutT_l = np.ascontiguousarray(
        np.broadcast_to(
            np.asarray(bout, np.float32).reshape(c.KD, P)
            .transpose(1, 0)[:, :, None],
            (P, c.KD, c.BL),
        ).astype(np.float32)
    )

    order, slot_npads = plan_slots(valid_mask, n_cores, c.BL)
    for bl in range(c.BL):
        assert slot_npads[bl] <= c.slot_npads[bl], (
            f"slot {bl}: cfg bound {c.slot_npads[bl]} < data {slot_npads[bl]}"
        )
    in_maps = []
    for core in range(n_cores):
        xt8_a = np.zeros((c.BL, P, c.KDR, 2, c.N_pad), f8)
        xlo_a = np.zeros((c.BL, P, c.KDR, 2, c.N_pad), f8)
        xc_a = np.zeros((c.BL, c.N_pad, D), f16)
        zcn_a = np.zeros((c.H, c.BL), np.float32)
        for bl in range(c.BL):
            bg = int(order[bl * n_cores + core])
            idx = np.where(valid_mask[bg])[0]
            cnt = len(idx)
            assert cnt <= c.slot_npads[bl]
            xv = np.asarray(x[bg][idx], np.float32)        # [cnt, D]
            x8 = xv.astype(f8)
            xr = xv - x8.astype(np.float32)
            xlo = xr.astype(f8)
            # [cnt, D] -> [P, D//256, 2, cnt]
            xt = x8.T.reshape(c.KDR, 2, P, cnt).transpose(2, 0, 1, 3)
            xl = xlo.T.reshape(c.KDR, 2, P, cnt).transpose(2, 0, 1, 3)
            xt8_a[bl, :, :, :, :cnt] = xt
            xlo_a[bl, :, :, :, :cnt] = xl
            xc_a[bl, :cnt] = xv.astype(f16)
            zcn_a[:, bl] = -(c.slot_npads[bl] - cnt) * e_pad
        in_maps.append({
            "xt8": np.ascontiguousarray(xt8_a),
            "xlo": np.ascontiguousarray(xlo_a),
            "xc": np.ascontiguousarray(xc_a),
            "w18": w18_l, "w1lo": w1lo_l,
            "w28": w28_l, "w28lo": w28lo_l, "b1T": b1T_l,
            "zcorrn": zcn_a,
            "wout": wout_l, "boutT": boutT_l,
        })
    return in_maps


_cached = {}
last_results = None


def kernel(x, valid_mask, W1, b1, W2, b2, Wout, bout, trace=False):
    global last_results
    x, valid_mask, W1, b1, W2, b2, Wout, bout = (
        np.asarray(a)
        for a in (x, valid_mask, W1, b1, W2, b2, Wout, bout)
    )
    B = x.shape[0]
    n_cores = 8
    BL = B // n_cores
    order, slot_npads = plan_slots(valid_mask, n_cores, BL)
    n_pad = max(slot_npads)
    cfg = Cfg(BL=BL, N_pad=n_pad, slot_npads=slot_npads)
    key = (B, n_pad, tuple(slot_npads))
    if key not in _cached:
        nc = bacc.Bacc("TRN2", target_bir_lowering=False, debug=False)
        build_kernel(nc, cfg)
        nc.compile()
        _cached[key] = nc
    in_maps = make_in_maps(x, valid_mask, W1, b1, W2, b2, Wout, bout,
                           n_cores, cfg)
    res = run_bass_kernel_spmd(
        _cached[key], in_maps, core_ids=list(range(n_cores)), trace=trace
    )
    last_results = res
    y = np.empty((B, cfg.D), np.float32)
    for core in range(n_cores):
        yT = np.asarray(res.results[core]["y"], np.float32)  # [P, KD, BL]
        yc = yT.transpose(2, 1, 0).reshape(BL, cfg.D)
        for bl in range(BL):
            y[int(order[bl * n_cores + core])] = yc[bl]
    return y


# revision 51
# speedup vs baseline: 1.0065x; 1.0065x over previous
"""AttentionPool Trainium2 Bass kernel (w2-mass-aware precision, fp8 DR).

Reference computation (per batch b):
    h      = tanh(x @ W1 + b1)          # [N, H*F]
    scores = h @ W2 + b2                # [N, H]   (b2 cancels under softmax)
    scores = where(mask, scores, -1e9)
    w      = softmax(scores, axis=N)
    pooled = w.T @ x                    # [H, D]
    y      = concat_h(pooled) @ Wout + bout

Strategy (vs the 151us 8-resid-pass baseline):
 1. Host-side valid-token compaction (~50% of tokens masked) and batch
    count-sorting into (core, slot), as before.
 2. Each head's F=512 hidden dims are PERMUTED so large-|w2| dims come
    first.  Per head, chunk 0 (128 dims) carries ~72% of sum(w2^2),
    chunk 1 ~20%, chunks 2-3 ~7%.  Score noise scales with the w2^2
    mass of the chunk it enters through, so precision is allocated by
    chunk rank:
      - main x8@w18 fp8 DoubleRow pass: all 16 mc chunks (irreducible)
      - residual passes ONLY on the 4 rank-0 chunks: x-residual on
        d>=256 (xlo@w18) + W-residual on d>=512 (x8@w1lo)
      - score dot h@W2 entirely in fp8 DoubleRow (tanh emits fp8
        directly); top pairs (ranks 0,1) get a w2-residual second pass.
    48 PE-cycles/token vs baseline's 80.  numpy-sim absmax 1.59e-2,
    HW-measured 1.57e-2 (gate 2e-2; baseline measured 1.52e-2).
 3. b1 is applied exactly as a per-partition fp32 bias AP in the tanh
    activation (tanh runs per-mc chunk), replacing the ones-row trick.
    b2 cancels under softmax.  w2 is pre-scaled by 8 (power of two,
    exact) so fp8 quantization stays out of subnormals; the exp
    activation un-scales with scale=1/8.
 4. NO pad mask: pad token columns are all-zero in xt8/xlo, so their
    h8 is exactly q8(tanh(b1)) and their pooling contribution is 0
    (xc rows are zero).  They only inflate the softmax denominator by
    (npad-cnt)*e^(s_pad), which the host computes exactly and the
    device subtracts (zcorrn input).  exp reads the score PSUM
    directly per block (accum per block), killing the DVE mask-add
    and the m16 tensor.
 5. Token blocks of TB=1024 (PSUM [128,1024] tiles, ring of 3).  The
    score dot is software-pipelined two pairs behind the h chains so
    the PE never waits on the tanh of the pair it just produced.
 6. The whole pooling path (e weights, x, Wout, pooled) runs fp16.

Layouts (d = dc*256 + i*128 + p for DoubleRow pairs; f = mc*128 + p
with mc = 4*head + rank after the per-head w2-sort):
  xt8  [BL, P, 4, 2, N_pad] fp8   x compacted, transposed, e4m3
  xlo  [BL, P, 4, 2, N_pad] fp8   q8(x - x8), all d
  xc   [BL, N_pad, D]      fp16   natural x for pooling
  w18  [P, 4, 2, HF] fp8          q8(8*W1f)  (W1f column-permuted)
  w1lo [P, 4, 2, 4*128] fp8       q8(8*W1f - w18), rank-0 cols, head-major
  w28  [P, 8, 2, 16] fp8          q8(8*w2) one-hot-by-head, DR pairs
                                  (head axis padded 4->16: DR LdWeights
                                  needs pair-axis byte-step % 16 == 0)
  w28lo[P, 4, 2, 16] fp8          q8(8*w2 - w28) for top pairs
  b1T  [P, 16] fp32               b1 per (p, mc), tanh bias APs
  zcorrn [H, BL] fp32             -(npad-cnt)*e^(s_pad) denominator fix
  wout [P, 32, D] fp16, boutT [P, 8, BL] fp32, y [P, 8, BL] fp32
"""

import numpy as np
import ml_dtypes

import concourse.bass as bass
import concourse.mybir as mybir
import concourse.tile as tile
from concourse import bacc
from concourse.bass import ts
from concourse.bass_utils import run_bass_kernel_spmd
from concourse.masks import make_identity

FP32 = mybir.dt.float32
F8 = mybir.dt.float8e4
FP16 = mybir.dt.float16
AFT = mybir.ActivationFunctionType
DR = mybir.MatmulPerfMode.DoubleRow

P = 128
TB = 1024


def _blocks(n_pad, tb=TB):
    """Split a slot into token blocks.  Oversize slots split into two
    BALANCED blocks (not 1024+tail): the Act engine's fixed per-tanh
    overhead makes tiny tail blocks Act-bound, stalling the next slot
    on the PSUM ring.  54/46 beats 50/50 (swept): the larger first
    block gives the tanh stream catch-up room at the block boundary."""
    if n_pad <= tb:
        return [(0, n_pad)]
    assert n_pad <= 2 * tb
    b0 = (n_pad * 54 // 100 + 15) // 16 * 16
    return [(0, b0), (b0, n_pad - b0)]


class Cfg:
    def __init__(self, BL=4, N=2048, D=1024, H=4, F=512, N_pad=1152,
                 slot_npads=None):
        self.BL, self.N, self.D, self.H, self.F = BL, N, D, H, F
        self.HF = H * F
        self.N_pad = N_pad
        self.KDR = D // 256          # DoubleRow d-chunks (256 each)
        self.MC = self.HF // P       # h col chunks (16)
        self.PC = self.MC // 2       # score-dot pairs (8)
        self.KD = D // P             # 128-chunks of D
        self.KOUT = (H * D) // P     # contraction chunks of the out proj
        self.slot_npads = list(slot_npads) if slot_npads else [N_pad] * BL
        assert len(self.slot_npads) == BL
        assert max(self.slot_npads) <= N_pad
        self.slot_blocks = [_blocks(np_) for np_ in self.slot_npads]
        self.slot_chunks = []
        for np_ in self.slot_npads:
            ch, n0 = [], 0
            while n0 < np_:
                s = min(P, np_ - n0)
                ch.append((n0, s))
                n0 += s
            self.slot_chunks.append(ch)
        self.CHM = max(12, max(len(ch) for ch in self.slot_chunks))


def build_kernel(nc: bass.Bass, cfg: Cfg, reps: int = 1):
    c = cfg
    xt8_d = nc.dram_tensor("xt8", [c.BL, P, c.KDR, 2, c.N_pad], F8,
                           kind="ExternalInput").ap()
    xlo_d = nc.dram_tensor("xlo", [c.BL, P, c.KDR, 2, c.N_pad], F8,
                           kind="ExternalInput").ap()
    xc_d = nc.dram_tensor("xc", [c.BL, c.N_pad, c.D], FP16,
                          kind="ExternalInput").ap()
    w18_d = nc.dram_tensor("w18", [P, c.KDR, 2, c.HF], F8,
                           kind="ExternalInput").ap()
    w1lo_d = nc.dram_tensor("w1lo", [P, c.KDR, 2, c.H * P], F8,
                            kind="ExternalInput").ap()
    w28_d = nc.dram_tensor("w28", [P, c.PC, 2, 16], F8,
                           kind="ExternalInput").ap()
    w28lo_d = nc.dram_tensor("w28lo", [P, c.PC // 2, 2, 16], F8,
                             kind="ExternalInput").ap()
    b1T_d = nc.dram_tensor("b1T", [P, c.MC], FP32,
                           kind="ExternalInput").ap()
    zcn_d = nc.dram_tensor("zcorrn", [c.H, c.BL], FP32,
                           kind="ExternalInput").ap()
    wout_d = nc.dram_tensor("wout", [P, c.KOUT, c.D], FP16,
                            kind="ExternalInput").ap()
    boutT_d = nc.dram_tensor("boutT", [P, c.KD, c.BL], FP32,
                             kind="ExternalInput").ap()
    y_d = nc.dram_tensor("y", [P, c.KD, c.BL], FP32,
                         kind="ExternalOutput").ap()

    with tile.TileContext(nc) as tc:
        with (
            tc.tile_pool(name="const", bufs=1) as const,
            tc.tile_pool(name="xt8p", bufs=3) as xt8_pool,
            tc.tile_pool(name="xlop", bufs=3) as xlo_pool,
            tc.tile_pool(name="h8p", bufs=12) as h8_pool,
            tc.tile_pool(name="xnp", bufs=20) as xn_pool,
            tc.tile_pool(name="ep", bufs=2) as e_pool,
            tc.tile_pool(name="etp", bufs=2) as eT_pool,
            tc.tile_pool(name="smallp", bufs=4) as small_pool,
            tc.tile_pool(name="hps", bufs=3, space="PSUM") as hps_pool,
            tc.tile_pool(name="scps", bufs=1, space="PSUM") as scps_pool,
        ):
            # ---- constants.  scalar/Act queue carries NO DMAs: the
            # issuing engine is blocked for the whole transfer in this
            # model, and Act is the co-bottleneck (tanh).  w18 streams in
            # pair-column slices ordered by first use (odd pairs first,
            # matching block 0's pc_order); b1T leads for the first tanh.
            # x-data rides the SP queue. ----
            w18_sb = const.tile([P, c.KDR, 2, c.HF], F8)
            w1lo_sb = const.tile([P, c.KDR, 2, c.H * P], F8)
            w28_sb = const.tile([P, c.PC, 2, 16], F8)
            w28lo_sb = const.tile([P, c.PC // 2, 2, 16], F8)
            b1T_sb = const.tile([P, c.MC], FP32)
            zcn_sb = const.tile([c.H, c.BL], FP32)
            for pc in (1, 3):
                cs = ts(pc, 256)
                nc.gpsimd.dma_start(w18_sb[:, :, :, cs], w18_d[:, :, :, cs])
            nc.gpsimd.dma_start(b1T_sb[:], b1T_d)
            for pc in (5, 7):
                cs = ts(pc, 256)
                nc.gpsimd.dma_start(w18_sb[:, :, :, cs], w18_d[:, :, :, cs])
            nc.gpsimd.dma_start(w1lo_sb[:], w1lo_d)
            nc.gpsimd.dma_start(w18_sb[:, :, :, ts(0, 256)],
                                w18_d[:, :, :, ts(0, 256)])
            nc.gpsimd.dma_start(w28_sb[:], w28_d)
            nc.gpsimd.dma_start(w28lo_sb[:], w28lo_d)
            for pc in (2, 4, 6):
                cs = ts(pc, 256)
                nc.gpsimd.dma_start(w18_sb[:, :, :, cs], w18_d[:, :, :, cs])
            nc.gpsimd.dma_start(zcn_sb[:], zcn_d)
            # wout, boutT are DMA'd from inside the first block's emission
            # so they queue behind the startup-critical transfers
            boutT_sb = const.tile([P, c.KD, c.BL], FP32)
            wout_sb = const.tile([P, c.KOUT, c.D], FP16)
            idH = const.tile([c.H, c.H], FP16)
            make_identity(nc, idH[:])
            idHf = const.tile([c.H, c.H], FP32)
            make_identity(nc, idHf[:])
            ones4 = const.tile([c.H, P], FP32)
            nc.vector.memset(ones4[:], 1.0)
            # warm the activation table (Tanh/Exp share one set) during
            # the initial DMA wait
            warm = const.tile([1, 1], FP32)
            nc.scalar.activation(warm[:], idH[:1, :1], AFT.Tanh)
            poolT_sb = const.tile([P, c.KD, c.H, c.BL], FP16)
            y_sbT = const.tile([P, c.KD, c.BL], FP32)

            def emit_xn_dmas(b):
                tiles = []
                for n0, cs in c.slot_chunks[b]:
                    xn = xn_pool.tile([P, c.D], FP16, tag="xn")
                    nc.gpsimd.dma_start(xn[:cs], xc_d[b, n0:n0 + cs, :])
                    tiles.append(xn)
                return tiles

            def emit_scores(b, rep):
                """DMA + h matmuls + tanh(fp8 out) + fp8-DR score dot +
                per-block exp straight off the score PSUM."""
                # +128 columns so a partial tail chunk transposes as a
                # full 128-wide chunk (zeroed here, off the critical path)
                e_bf = e_pool.tile([c.H, c.N_pad + 128], FP16, tag="e")
                zsb = small_pool.tile([c.H, len(c.slot_blocks[b])],
                                      FP32, tag="zsb")
                nbv = c.slot_npads[b]
                padv = (-nbv) % P
                if padv:
                    nc.vector.memset(e_bf[:, nbv:nbv + padv], 0.0)
                first = b == 0 and rep == 0
                for blk, (n0, tb) in enumerate(c.slot_blocks[b]):
                    xt8 = xt8_pool.tile([P, c.KDR, 2, TB], F8, tag="xt8")
                    xlo = xlo_pool.tile([P, c.KDR, 2, TB], F8, tag="xlo")
                    if first and blk == 0:
                        # per-dc split so the first chains start as soon
                        # as their own chunk lands
                        for dc in range(c.KDR):
                            nc.sync.dma_start(
                                xt8[:, dc, :, :tb],
                                xt8_d[b, :, dc, :, n0:n0 + tb],
                            )
                        for dc in range(c.KDR):
                            nc.sync.dma_start(
                                xlo[:, dc, :, :tb],
                                xlo_d[b, :, dc, :, n0:n0 + tb],
                            )
                        nc.gpsimd.dma_start(wout_sb[:], wout_d)
                        nc.gpsimd.dma_start(boutT_sb[:], boutT_d)
                    else:
                        nc.sync.dma_start(
                            xt8[:, :, :, :tb], xt8_d[b, :, :, :, n0:n0 + tb]
                        )
                        nc.sync.dma_start(
                            xlo[:, :, :, :tb], xlo_d[b, :, :, :, n0:n0 + tb]
                        )
                    # first block: residual-free odd pairs first so the
                    # PE isn't gated on the xlo DMA splits at startup
                    if first and blk == 0:
                        pc_order = [1, 3, 5, 7, 0, 2, 4, 6]
                    else:
                        pc_order = list(range(c.PC))
                    h8s = [None] * c.PC
                    hv = [(s0, min(512, tb - s0)) for s0 in range(0, tb, 512)]
                    scp = scps_pool.tile([16, TB], FP32, tag="scps")
                    # each 512-col PSUM half is its own accumulation
                    # group: first/last instr per half carry start/stop
                    _per_half = c.PC + c.PC // 2
                    n_sc = [0] * len(hv)

                    def emit_score(pc):
                        for hi, (s0, sw) in enumerate(hv):
                            sl = slice(s0, s0 + sw)
                            nc.tensor.matmul(
                                scp[:, sl], w28_sb[:, pc],
                                h8s[pc][:, :, sl],
                                start=(n_sc[hi] == 0),
                                stop=(n_sc[hi] == _per_half - 1),
                                perf_mode=DR,
                            )
                            n_sc[hi] += 1
                            if pc % 2 == 0:
                                nc.tensor.matmul(
                                    scp[:, sl], w28lo_sb[:, pc // 2],
                                    h8s[pc][:, :, sl],
                                    start=False,
                                    stop=(n_sc[hi] == _per_half - 1),
                                    perf_mode=DR,
                                )
                                n_sc[hi] += 1
                    pend = []
                    for pc in pc_order:
                        h8 = h8_pool.tile([P, 2, TB], F8, tag="h8")
                        top = pc % 2 == 0   # ranks {0,1} pair of head pc//2
                        hd = pc // 2
                        for j in range(2):
                            mc = 2 * pc + j
                            r0 = top and j == 0  # rank-0 chunk
                            ms = ts(mc, P)
                            hp = hps_pool.tile([P, TB], FP32, tag="h")
                            for s0, sw in hv:
                                sl = slice(s0, s0 + sw)
                                for dc in range(c.KDR):
                                    nc.tensor.matmul(
                                        hp[:, sl],
                                        w18_sb[:, dc, :, ms],
                                        xt8[:, dc, :, sl],
                                        start=(dc == 0),
                                        stop=(dc == c.KDR - 1 and not r0),
                                        perf_mode=DR,
                                    )
                                if r0:
                                    # x-resid on d>=256 + W-resid d>=512
                                    # (the d<256 quarter is dropped: its
                                    # noise share is small vs 2 cyc/tok)
                                    for dc in (1, 2, 3):
                                        nc.tensor.matmul(
                                            hp[:, sl],
                                            w18_sb[:, dc, :, ms],
                                            xlo[:, dc, :, sl],
                                            start=False, stop=False,
                                            perf_mode=DR,
                                        )
                                    for dc in (2, 3):
                                        nc.tensor.matmul(
                                            hp[:, sl],
                                            w1lo_sb[:, dc, :, ts(hd, P)],
                                            xt8[:, dc, :, sl],
                                            start=False, stop=(dc == 3),
                                            perf_mode=DR,
                                        )
                            nc.scalar.activation(
                                h8[:, j, :tb], hp[:, :tb], AFT.Tanh,
                                bias=b1T_sb[:, mc:mc + 1], scale=0.125,
                            )
                        h8s[pc] = h8
                        # score dot runs two pairs behind the chains so
                        # the PE never waits on the tanh it just fed
                        pend.append(pc)
                        lag = 8 if (first and blk > 0) else 4
                        if len(pend) > lag:
                            emit_score(pend.pop(0))
                    for pc in pend:
                        emit_score(pc)
                    # per-block exp straight off the score PSUM (scores
                    # are at 8x scale; pad tokens included, fixed via
                    # zcorrn in the softmax denominator)
                    nc.scalar.activation(
                        e_bf[:, n0:n0 + tb], scp[:c.H, :tb],
                        AFT.Exp, bias=0.0, scale=0.125,
                        accum_out=zsb[:, blk:blk + 1],
                    )
                xn_tiles = emit_xn_dmas(b)
                return (e_bf, zsb), xn_tiles

            def emit_tail(b, e_zsb, xn_tiles):
                """softmax + pooling for batch b (runs under b+1's scores).

                Pooling contracts the RAW (unnormalized) e values; the
                1/Z per-head scale is applied to the tiny pooled [D, H]
                matrix at the end, so the transpose/pool chain never
                waits on the denominator.  Z reaches all 128 partitions
                via ones.T @ (idH * zs).
                """
                e_bf, zsb = e_zsb
                zs = small_pool.tile([c.H, 1], FP32, tag="zs")
                # denominator: sum block accums, subtract the host-computed
                # pad contribution (zcorrn is negated on the host)
                nc.vector.tensor_add(
                    zs[:], zsb[:, 0:1], zcn_sb[:, b:b + 1]
                )
                for blk in range(1, len(c.slot_blocks[b])):
                    nc.vector.tensor_add(zs[:], zs[:], zsb[:, blk:blk + 1])
                zdiag = small_pool.tile([c.H, c.H], FP32, tag="zdiag")
                nc.vector.tensor_scalar_mul(zdiag[:], idHf[:], zs[:])
                chunks = c.slot_chunks[b]
                ncb = len(chunks)
                tpt = hps_pool.tile([P, c.CHM, c.H], FP16, tag="h")
                for cn, (n0, cs) in enumerate(chunks):
                    nc.tensor.matmul(
                        tpt[:, cn, :], e_bf[:, n0:n0 + P], idH[:],
                        is_transpose=True,
                        start=(cn == 0), stop=(cn == ncb - 1),
                    )
                eTt = eT_pool.tile([P, c.CHM, c.H], FP16, tag="eT")
                nc.vector.tensor_copy(eTt[:, :ncb], tpt[:, :ncb])
                zbc = hps_pool.tile([P, c.H], FP32, tag="h")
                nc.tensor.matmul(zbc[:], ones4[:], zdiag[:],
                                 start=True, stop=True)
                rzbc = small_pool.tile([P, c.H], FP32, tag="rzbc")
                nc.vector.reciprocal(rzbc[:], zbc[:])
                plt = hps_pool.tile([P, c.KD, c.H], FP32, tag="h")
                for dc in range(c.KD):
                    for cn, (n0, cs) in enumerate(chunks):
                        nc.tensor.matmul(
                            plt[:, dc, :], xn_tiles[cn][:cs, ts(dc, P)],
                            eTt[:cs, cn, :],
                            start=(cn == 0), stop=(cn == ncb - 1),
                        )
                for hd in range(c.H):
                    nc.vector.tensor_scalar_mul(
                        poolT_sb[:, :, hd, b], plt[:, :, hd],
                        rzbc[:, hd:hd + 1],
                    )

            for rep in range(reps):
                prev = None
                for b in range(c.BL):
                    e_zsb, xn_tiles = emit_scores(b, rep)
                    if prev is not None:
                        emit_tail(prev[0], prev[1], prev[2])
                    prev = (b, e_zsb, xn_tiles)
                emit_tail(prev[0], prev[1], prev[2])
                # out projection for ALL batch columns in one pass
                # (4-col moving operands quarter the instruction count)
                ytp = hps_pool.tile([P, c.KD, c.BL], FP32, tag="h")
                for dblk in range(c.KD):
                    for kc in range(c.KOUT):
                        hd, dc = kc // c.KD, kc % c.KD
                        nc.tensor.matmul(
                            ytp[:, dblk, :],
                            wout_sb[:, kc, ts(dblk, P)],
                            poolT_sb[:, dc, hd, :],
                            start=(kc == 0), stop=(kc == c.KOUT - 1),
                        )
                nc.vector.tensor_add(y_sbT[:], ytp[:], boutT_sb[:])
                nc.sync.dma_start(y_d[:], y_sbT[:])
    return nc


def plan_slots(valid_mask, n_cores, BL):
    """Count-sort batches into (core, slot) so each SPMD batch-slot has a
    tight shared token bound."""
    counts = np.asarray(valid_mask).sum(1)
    order = np.argsort(counts, kind="stable")[::-1]
    slot_npads = []
    for bl in range(BL):
        grp = order[bl * n_cores:(bl + 1) * n_cores]
        mx = int(counts[grp].max())
        slot_npads.append(max(256, int(np.ceil(mx / 16) * 16)))
    return order, slot_npads


def make_in_maps(x, valid_mask, W1, b1, W2, b2, Wout, bout, n_cores, cfg):
    """Host-side prep: w2-sort heads' dims, compact valid tokens, fp8
    layouts, shard over batch."""
    c = cfg
    f8 = ml_dtypes.float8_e4m3
    f16 = np.float16
    B, N, D = x.shape
    H, _, F = W1.shape
    HF = H * F

    def q8(a):
        return np.asarray(a, np.float32).astype(f8)

    # per-head permutation: large |w2| dims first
    perm = [np.argsort(-np.abs(np.asarray(W2[h], np.float32)),
                       kind="stable") for h in range(H)]
    W1p = np.stack([np.asarray(W1[h], np.float32)[:, perm[h]]
                    for h in range(H)])          # [H, D, F]
    b1p = np.stack([np.asarray(b1[h], np.float32)[perm[h]]
                    for h in range(H)])          # [H, F]
    w2p = np.stack([np.asarray(W2[h], np.float32)[perm[h]]
                    for h in range(H)])          # [H, F]

    W1f = W1p.transpose(1, 0, 2).reshape(D, HF)  # [D, HF] head-major cols
    w18 = q8(8.0 * W1f)
    w1r = 8.0 * W1f - w18.astype(np.float32)

    def dr_pack_w(wmat):
        # [D, M] -> [P, D//256, 2, M] with d = dc*256 + i*128 + p
        Dw, M = wmat.shape
        return np.ascontiguousarray(
            wmat.reshape(Dw // 256, 2, P, M).transpose(2, 0, 1, 3)
        )

    w18_l = dr_pack_w(w18)
    # rank-0 columns of each head, head-major compact [D, H*P]
    r0cols = np.concatenate(
        [np.arange(4 * h * P, (4 * h + 1) * P) for h in range(H)]
    )
    w1lo_l = dr_pack_w(q8(w1r[:, r0cols]))

    # score-dot stationaries: w28[p, pc, j, hd] = q8(8*w2[f]) one-hot by
    # head, f = (2*pc+j)*128 + p, head = (2*pc+j)//4.  Head axis padded
    # 4->16 for the DR LdWeights pair-step%16 rule.
    w2f8 = q8(8.0 * w2p.reshape(HF))
    w2flo = q8(8.0 * w2p.reshape(HF) - w2f8.astype(np.float32))
    w28_l = np.zeros((P, c.PC, 2, 16), f8)
    w28lo_l = np.zeros((P, c.PC // 2, 2, 16), f8)
    for pc in range(c.PC):
        for j in range(2):
            mc = 2 * pc + j
            fidx = mc * P + np.arange(P)
            w28_l[np.arange(P), pc, j, mc // 4] = w2f8[fidx]
            if pc % 2 == 0:
                w28lo_l[np.arange(P), pc // 2, j, mc // 4] = w2flo[fidx]
    w28_l = np.ascontiguousarray(w28_l)
    w28lo_l = np.ascontiguousarray(w28lo_l)

    b1T_l = np.ascontiguousarray(
        b1p.reshape(HF).reshape(c.MC, P).transpose(1, 0).astype(np.float32)
    )

    # pad-token score per head at the device's exact precision:
    # h_pad = e4m3(tanh(b1)), s_pad8 = sum_f w28[f]*h_pad[f] (+ w28lo on
    # top pairs); e_pad = exp(s_pad8/8)
    hpadq = np.asarray(np.tanh(b1p.reshape(HF)), np.float32).astype(f8)
    hpadf = hpadq.astype(np.float32)
    w28f = w2f8.astype(np.float32)
    w28lof = w2flo.astype(np.float32)
    s_pad8 = np.zeros(H, np.float64)
    for mc in range(c.MC):
        fidx = mc * P + np.arange(P)
        hd = mc // 4
        s_pad8[hd] += (w28f[fidx] * hpadf[fidx]).sum()
        if (mc // 2) % 2 == 0:   # top pair -> w2lo residual applies
            s_pad8[hd] += (w28lof[fidx] * hpadf[fidx]).sum()
    e_pad = np.exp(s_pad8 / 8.0)                 # [H]

    wout_l = np.ascontiguousarray(
        np.asarray(Wout, np.float32).reshape(c.KOUT, P, c.D)
        .transpose(1, 0, 2).astype(f16)
    )
    boutT_l = np.ascontiguousarray(
        np.broadcast_to(
            np.asarray(bout, np.float32).reshape(c.KD, P)
            .transpose(1, 0)[:, :, None],
            (P, c.KD, c.BL),
        ).astype(np.float32)
    )

    order, slot_npads = plan_slots(valid_mask, n_cores, c.BL)
    for bl in range(c.BL):
        assert slot_npads[bl] <= c.slot_npads[bl], (
            f"slot {bl}: cfg bound {c.slot_npads[bl]} < data {slot_npads[bl]}"
        )
    in_maps = []
    for core in range(n_cores):
        xt8_a = np.zeros((c.BL, P, c.KDR, 2, c.N_pad), f8)
        xlo_a = np.zeros((c.BL, P, c.KDR, 2, c.N_pad), f8)
        xc_a = np.zeros((c.BL, c.N_pad, D), f16)
        zcn_a = np.zeros((c.H, c.BL), np.float32)
        for bl in range(c.BL):
            bg = int(order[bl * n_cores + core])
            idx = np.where(valid_mask[bg])[0]
            cnt = len(idx)
            assert cnt <= c.slot_npads[bl]
            xv = np.asarray(x[bg][idx], np.float32)        # [cnt, D]
            x8 = xv.astype(f8)
            xr = xv - x8.astype(np.float32)
            xlo = xr.astype(f8)
            # [cnt, D] -> [P, D//256, 2, cnt]
            xt = x8.T.reshape(c.KDR, 2, P, cnt).transpose(2, 0, 1, 3)
            xl = xlo.T.reshape(c.KDR, 2, P, cnt).transpose(2, 0, 1, 3)
            xt8_a[bl, :, :, :, :cnt] = xt
            xlo_a[bl, :, :, :, :cnt] = xl
            xc_a[bl, :cnt] = xv.astype(f16)
            zcn_a[:, bl] = -(c.slot_npads[bl] - cnt) * e_pad
        in_maps.append({
            "xt8": np.ascontiguousarray(xt8_a),
            "xlo": np.ascontiguousarray(xlo_a),
            "xc": np.ascontiguousarray(xc_a),
            "w18": w18_l, "w1lo": w1lo_l,
            "w28": w28_l, "w28lo": w28lo_l, "b1T": b1T_l,
            "zcorrn": zcn_a,
            "wout": wout_l, "boutT": boutT_l,
        })
    return in_maps


_cached = {}
last_results = None


def kernel(x, valid_mask, W1, b1, W2, b2, Wout, bout, trace=False):
    global last_results
    x, valid_mask, W1, b1, W2, b2, Wout, bout = (
        np.asarray(a)
        for a in (x, valid_mask, W1, b1, W2, b2, Wout, bout)
    )
    B = x.shape[0]
    n_cores = 8
    BL = B // n_cores
    order, slot_npads = plan_slots(valid_mask, n_cores, BL)
    n_pad = max(slot_npads)
    cfg = Cfg(BL=BL, N_pad=n_pad, slot_npads=slot_npads)
    key = (B, n_pad, tuple(slot_npads))
    if key not in _cached:
        nc = bacc.Bacc("TRN2", target_bir_lowering=False, debug=False)
        build_kernel(nc, cfg)
        nc.compile()
        _cached[key] = nc
    in_maps = make_in_maps(x, valid_mask, W1, b1, W2, b2, Wout, bout,
                           n_cores, cfg)
    res = run_bass_kernel_spmd(
        _cached[key], in_maps, core_ids=list(range(n_cores)), trace=trace
    )
    last_results = res
    y = np.empty((B, cfg.D), np.float32)
    for core in range(n_cores):
        yT = np.asarray(res.results[core]["y"], np.float32)  # [P, KD, BL]
        yc = yT.transpose(2, 1, 0).reshape(BL, cfg.D)
        for bl in range(BL):
            y[int(order[bl * n_cores + core])] = yc[bl]
    return y


# revision 57
# speedup vs baseline: 1.0072x; 1.0006x over previous
"""AttentionPool Trainium2 Bass kernel (w2-mass-aware precision, fp8 DR).

Reference computation (per batch b):
    h      = tanh(x @ W1 + b1)          # [N, H*F]
    scores = h @ W2 + b2                # [N, H]   (b2 cancels under softmax)
    scores = where(mask, scores, -1e9)
    w      = softmax(scores, axis=N)
    pooled = w.T @ x                    # [H, D]
    y      = concat_h(pooled) @ Wout + bout

Strategy (vs the 151us 8-resid-pass baseline):
 1. Host-side valid-token compaction (~50% of tokens masked) and batch
    count-sorting into (core, slot), as before.
 2. Each head's F=512 hidden dims are PERMUTED so large-|w2| dims come
    first.  Per head, chunk 0 (128 dims) carries ~72% of sum(w2^2),
    chunk 1 ~20%, chunks 2-3 ~7%.  Score noise scales with the w2^2
    mass of the chunk it enters through, so precision is allocated by
    chunk rank:
      - main x8@w18 fp8 DoubleRow pass: all 16 mc chunks (irreducible)
      - residual passes ONLY on the 4 rank-0 chunks: x-residual on
        d>=256 (xlo@w18) + W-residual on d>=512 (x8@w1lo)
      - score dot h@W2 entirely in fp8 DoubleRow (tanh emits fp8
        directly); top pairs (ranks 0,1) get a w2-residual second pass.
    48 PE-cycles/token vs baseline's 80.  numpy-sim absmax 1.59e-2,
    HW-measured 1.57e-2 (gate 2e-2; baseline measured 1.52e-2).
 3. b1 is applied exactly as a per-partition fp32 bias AP in the tanh
    activation (tanh runs per-mc chunk), replacing the ones-row trick.
    b2 cancels under softmax.  w2 is pre-scaled by 8 (power of two,
    exact) so fp8 quantization stays out of subnormals; the exp
    activation un-scales with scale=1/8.
 4. NO pad mask: pad token columns are all-zero in xt8/xlo, so their
    h8 is exactly q8(tanh(b1)) and their pooling contribution is 0
    (xc rows are zero).  They only inflate the softmax denominator by
    (npad-cnt)*e^(s_pad), which the host computes exactly and the
    device subtracts (zcorrn input).  exp reads the score PSUM
    directly per block (accum per block), killing the DVE mask-add
    and the m16 tensor.
 5. Token blocks of TB=1024 (PSUM [128,1024] tiles, ring of 3).  The
    score dot is software-pipelined two pairs behind the h chains so
    the PE never waits on the tanh of the pair it just produced.
 6. The whole pooling path (e weights, x, Wout, pooled) runs fp16.

Layouts (d = dc*256 + i*128 + p for DoubleRow pairs; f = mc*128 + p
with mc = 4*head + rank after the per-head w2-sort):
  xt8  [BL, P, 4, 2, N_pad] fp8   x compacted, transposed, e4m3
  xlo  [BL, P, 4, 2, N_pad] fp8   q8(x - x8), all d
  xc   [BL, N_pad, D]      fp16   natural x for pooling
  w18  [P, 4, 2, HF] fp8          q8(8*W1f)  (W1f column-permuted)
  w1lo [P, 4, 2, 4*128] fp8       q8(8*W1f - w18), rank-0 cols, head-major
  w28  [P, 8, 2, 16] fp8          q8(8*w2) one-hot-by-head, DR pairs
                                  (head axis padded 4->16: DR LdWeights
                                  needs pair-axis byte-step % 16 == 0)
  w28lo[P, 4, 2, 16] fp8          q8(8*w2 - w28) for top pairs
  b1T  [P, 16] fp32               b1 per (p, mc), tanh bias APs
  zcorrn [H, BL] fp32             -(npad-cnt)*e^(s_pad) denominator fix
  wout [P, 32, D] fp16, boutT [P, 8, BL] fp32, y [P, 8, BL] fp32
"""

import numpy as np
import ml_dtypes

import concourse.bass as bass
import concourse.mybir as mybir
import concourse.tile as tile
from concourse import bacc
from concourse.bass import ts
from concourse.bass_utils import run_bass_kernel_spmd
from concourse.masks import make_identity

FP32 = mybir.dt.float32
F8 = mybir.dt.float8e4
FP16 = mybir.dt.float16
AFT = mybir.ActivationFunctionType
DR = mybir.MatmulPerfMode.DoubleRow

P = 128
TB = 1024


def _blocks(n_pad, tb=TB):
    """Split a slot into token blocks.  Oversize slots split into two
    BALANCED blocks (not 1024+tail): the Act engine's fixed per-tanh
    overhead makes tiny tail blocks Act-bound, stalling the next slot
    on the PSUM ring.  54/46 beats 50/50 (swept): the larger first
    block gives the tanh stream catch-up room at the block boundary."""
    if n_pad <= tb:
        return [(0, n_pad)]
    assert n_pad <= 2 * tb
    b0 = (n_pad * 54 // 100 + 15) // 16 * 16
    return [(0, b0), (b0, n_pad - b0)]


class Cfg:
    def __init__(self, BL=4, N=2048, D=1024, H=4, F=512, N_pad=1152,
                 slot_npads=None):
        self.BL, self.N, self.D, self.H, self.F = BL, N, D, H, F
        self.HF = H * F
        self.N_pad = N_pad
        self.KDR = D // 256          # DoubleRow d-chunks (256 each)
        self.MC = self.HF // P       # h col chunks (16)
        self.PC = self.MC // 2       # score-dot pairs (8)
        self.KD = D // P             # 128-chunks of D
        self.KOUT = (H * D) // P     # contraction chunks of the out proj
        self.slot_npads = list(slot_npads) if slot_npads else [N_pad] * BL
        assert len(self.slot_npads) == BL
        assert max(self.slot_npads) <= N_pad
        self.slot_blocks = [_blocks(np_) for np_ in self.slot_npads]
        self.slot_chunks = []
        for np_ in self.slot_npads:
            ch, n0 = [], 0
            while n0 < np_:
                s = min(P, np_ - n0)
                ch.append((n0, s))
                n0 += s
            self.slot_chunks.append(ch)
        self.CHM = max(12, max(len(ch) for ch in self.slot_chunks))


def build_kernel(nc: bass.Bass, cfg: Cfg, reps: int = 1):
    c = cfg
    xt8_d = nc.dram_tensor("xt8", [c.BL, P, c.KDR, 2, c.N_pad], F8,
                           kind="ExternalInput").ap()
    xlo_d = nc.dram_tensor("xlo", [c.BL, P, c.KDR, 2, c.N_pad], F8,
                           kind="ExternalInput").ap()
    xc_d = nc.dram_tensor("xc", [c.BL, c.N_pad, c.D], FP16,
                          kind="ExternalInput").ap()
    w18_d = nc.dram_tensor("w18", [P, c.KDR, 2, c.HF], F8,
                           kind="ExternalInput").ap()
    w1lo_d = nc.dram_tensor("w1lo", [P, c.KDR, 2, c.H * P], F8,
                            kind="ExternalInput").ap()
    w28_d = nc.dram_tensor("w28", [P, c.PC, 2, 16], F8,
                           kind="ExternalInput").ap()
    w28lo_d = nc.dram_tensor("w28lo", [P, c.PC // 2, 2, 16], F8,
                             kind="ExternalInput").ap()
    b1T_d = nc.dram_tensor("b1T", [P, c.MC], FP32,
                           kind="ExternalInput").ap()
    zcn_d = nc.dram_tensor("zcorrn", [c.H, c.BL], FP32,
                           kind="ExternalInput").ap()
    wout_d = nc.dram_tensor("wout", [P, c.KOUT, c.D], FP16,
                            kind="ExternalInput").ap()
    boutT_d = nc.dram_tensor("boutT", [P, c.KD, c.BL], FP32,
                             kind="ExternalInput").ap()
    y_d = nc.dram_tensor("y", [P, c.KD, c.BL], FP32,
                         kind="ExternalOutput").ap()

    with tile.TileContext(nc) as tc:
        with (
            tc.tile_pool(name="const", bufs=1) as const,
            tc.tile_pool(name="xt8p", bufs=3) as xt8_pool,
            tc.tile_pool(name="xlop", bufs=3) as xlo_pool,
            tc.tile_pool(name="h8p", bufs=12) as h8_pool,
            tc.tile_pool(name="xnp", bufs=20) as xn_pool,
            tc.tile_pool(name="ep", bufs=2) as e_pool,
            tc.tile_pool(name="etp", bufs=2) as eT_pool,
            tc.tile_pool(name="smallp", bufs=4) as small_pool,
            tc.tile_pool(name="hps", bufs=3, space="PSUM") as hps_pool,
            tc.tile_pool(name="scps", bufs=1, space="PSUM") as scps_pool,
        ):
            # ---- constants.  scalar/Act queue carries NO DMAs: the
            # issuing engine is blocked for the whole transfer in this
            # model, and Act is the co-bottleneck (tanh).  w18 streams in
            # pair-column slices ordered by first use (odd pairs first,
            # matching block 0's pc_order); b1T leads for the first tanh.
            # x-data rides the SP queue. ----
            w18_sb = const.tile([P, c.KDR, 2, c.HF], F8)
            w1lo_sb = const.tile([P, c.KDR, 2, c.H * P], F8)
            w28_sb = const.tile([P, c.PC, 2, 16], F8)
            w28lo_sb = const.tile([P, c.PC // 2, 2, 16], F8)
            b1T_sb = const.tile([P, c.MC], FP32)
            zcn_sb = const.tile([c.H, c.BL], FP32)
            for pc in (1, 3):
                cs = ts(pc, 256)
                nc.gpsimd.dma_start(w18_sb[:, :, :, cs], w18_d[:, :, :, cs])
            nc.gpsimd.dma_start(b1T_sb[:], b1T_d)
            for pc in (5, 7):
                cs = ts(pc, 256)
                nc.gpsimd.dma_start(w18_sb[:, :, :, cs], w18_d[:, :, :, cs])
            nc.gpsimd.dma_start(w1lo_sb[:], w1lo_d)
            nc.gpsimd.dma_start(w18_sb[:, :, :, ts(0, 256)],
                                w18_d[:, :, :, ts(0, 256)])
            nc.gpsimd.dma_start(w28_sb[:], w28_d)
            nc.gpsimd.dma_start(w28lo_sb[:], w28lo_d)
            for pc in (2, 4, 6):
                cs = ts(pc, 256)
                nc.gpsimd.dma_start(w18_sb[:, :, :, cs], w18_d[:, :, :, cs])
            nc.gpsimd.dma_start(zcn_sb[:], zcn_d)
            # wout, boutT are DMA'd from inside the first block's emission
            # so they queue behind the startup-critical transfers
            boutT_sb = const.tile([P, c.KD, c.BL], FP32)
            wout_sb = const.tile([P, c.KOUT, c.D], FP16)
            idH = const.tile([c.H, c.H], FP16)
            make_identity(nc, idH[:])
            idHf = const.tile([c.H, c.H], FP32)
            make_identity(nc, idHf[:])
            ones4 = const.tile([c.H, P], FP32)
            nc.vector.memset(ones4[:], 1.0)
            # warm the activation table (Tanh/Exp share one set) during
            # the initial DMA wait
            warm = const.tile([1, 1], FP32)
            nc.scalar.activation(warm[:], idH[:1, :1], AFT.Tanh)
            poolT_sb = const.tile([P, c.KD, c.H, c.BL], FP16)
            y_sbT = const.tile([P, c.KD, c.BL], FP32)

            def emit_xn_dmas(b):
                tiles = []
                for n0, cs in c.slot_chunks[b]:
                    xn = xn_pool.tile([P, c.D], FP16, tag="xn")
                    nc.gpsimd.dma_start(xn[:cs], xc_d[b, n0:n0 + cs, :])
                    tiles.append(xn)
                return tiles

            def emit_scores(b, rep):
                """DMA + h matmuls + tanh(fp8 out) + fp8-DR score dot +
                per-block exp straight off the score PSUM."""
                # +128 columns so a partial tail chunk transposes as a
                # full 128-wide chunk (zeroed here, off the critical path)
                e_bf = e_pool.tile([c.H, c.N_pad + 128], FP16, tag="e")
                zsb = small_pool.tile([c.H, len(c.slot_blocks[b])],
                                      FP32, tag="zsb")
                nbv = c.slot_npads[b]
                padv = (-nbv) % P
                if padv:
                    nc.vector.memset(e_bf[:, nbv:nbv + padv], 0.0)
                first = b == 0 and rep == 0
                for blk, (n0, tb) in enumerate(c.slot_blocks[b]):
                    xt8 = xt8_pool.tile([P, c.KDR, 2, TB], F8, tag="xt8")
                    xlo = xlo_pool.tile([P, c.KDR, 2, TB], F8, tag="xlo")
                    if first and blk == 0:
                        # per-dc split so the first chains start as soon
                        # as their own chunk lands
                        for dc in range(c.KDR):
                            nc.sync.dma_start(
                                xt8[:, dc, :, :tb],
                                xt8_d[b, :, dc, :, n0:n0 + tb],
                            )
                        for dc in range(c.KDR):
                            nc.sync.dma_start(
                                xlo[:, dc, :, :tb],
                                xlo_d[b, :, dc, :, n0:n0 + tb],
                            )
                        nc.gpsimd.dma_start(wout_sb[:], wout_d)
                        nc.gpsimd.dma_start(boutT_sb[:], boutT_d)
                    else:
                        nc.sync.dma_start(
                            xt8[:, :, :, :tb], xt8_d[b, :, :, :, n0:n0 + tb]
                        )
                        nc.sync.dma_start(
                            xlo[:, :, :, :tb], xlo_d[b, :, :, :, n0:n0 + tb]
                        )
                    # first block: residual-free odd pairs first so the
                    # PE isn't gated on the xlo DMA splits at startup
                    if first and blk == 0:
                        pc_order = [1, 3, 5, 7, 0, 2, 4, 6]
                    else:
                        pc_order = list(range(c.PC))
                    h8s = [None] * c.PC
                    hv = [(s0, min(512, tb - s0)) for s0 in range(0, tb, 512)]
                    scp = scps_pool.tile([16, TB], FP32, tag="scps")
                    # each 512-col PSUM half is its own accumulation
                    # group: first/last instr per half carry start/stop
                    _per_half = c.PC + c.PC // 2
                    n_sc = [0] * len(hv)

                    def emit_score(pc):
                        for hi, (s0, sw) in enumerate(hv):
                            sl = slice(s0, s0 + sw)
                            nc.tensor.matmul(
                                scp[:, sl], w28_sb[:, pc],
                                h8s[pc][:, :, sl],
                                start=(n_sc[hi] == 0),
                                stop=(n_sc[hi] == _per_half - 1),
                                perf_mode=DR,
                            )
                            n_sc[hi] += 1
                            if pc % 2 == 0:
                                nc.tensor.matmul(
                                    scp[:, sl], w28lo_sb[:, pc // 2],
                                    h8s[pc][:, :, sl],
                                    start=False,
                                    stop=(n_sc[hi] == _per_half - 1),
                                    perf_mode=DR,
                                )
                                n_sc[hi] += 1
                    pend = []
                    for pc in pc_order:
                        h8 = h8_pool.tile([P, 2, TB], F8, tag="h8")
                        top = pc % 2 == 0   # ranks {0,1} pair of head pc//2
                        hd = pc // 2
                        for j in range(2):
                            mc = 2 * pc + j
                            r0 = top and j == 0  # rank-0 chunk
                            ms = ts(mc, P)
                            hp = hps_pool.tile([P, TB], FP32, tag="h")
                            for s0, sw in hv:
                                sl = slice(s0, s0 + sw)
                                for dc in range(c.KDR):
                                    nc.tensor.matmul(
                                        hp[:, sl],
                                        w18_sb[:, dc, :, ms],
                                        xt8[:, dc, :, sl],
                                        start=(dc == 0),
                                        stop=(dc == c.KDR - 1 and not r0),
                                        perf_mode=DR,
                                    )
                                if r0:
                                    # x-resid on d>=256 + W-resid d>=512
                                    # (the d<256 quarter is dropped: its
                                    # noise share is small vs 2 cyc/tok)
                                    for dc in (1, 2, 3):
                                        nc.tensor.matmul(
                                            hp[:, sl],
                                            w18_sb[:, dc, :, ms],
                                            xlo[:, dc, :, sl],
                                            start=False, stop=False,
                                            perf_mode=DR,
                                        )
                                    for dc in (2, 3):
                                        nc.tensor.matmul(
                                            hp[:, sl],
                                            w1lo_sb[:, dc, :, ts(hd, P)],
                                            xt8[:, dc, :, sl],
                                            start=False, stop=(dc == 3),
                                            perf_mode=DR,
                                        )
                            nc.scalar.activation(
                                h8[:, j, :tb], hp[:, :tb], AFT.Tanh,
                                bias=b1T_sb[:, mc:mc + 1], scale=0.125,
                            )
                        h8s[pc] = h8
                        # score dot runs two pairs behind the chains so
                        # the PE never waits on the tanh it just fed
                        pend.append(pc)
                        lag = 8 if (first and blk > 0) else 4
                        if len(pend) > lag:
                            emit_score(pend.pop(0))
                    for pc in pend:
                        emit_score(pc)
                    # per-block exp straight off the score PSUM (scores
                    # are at 8x scale; pad tokens included, fixed via
                    # zcorrn in the softmax denominator)
                    nc.scalar.activation(
                        e_bf[:, n0:n0 + tb], scp[:c.H, :tb],
                        AFT.Exp, bias=0.0, scale=0.125,
                        accum_out=zsb[:, blk:blk + 1],
                    )
                xn_tiles = emit_xn_dmas(b)
                return (e_bf, zsb), xn_tiles

            def emit_tail(b, e_zsb, xn_tiles):
                """softmax + pooling for batch b (runs under b+1's scores).

                Pooling contracts the RAW (unnormalized) e values; the
                1/Z per-head scale is applied to the tiny pooled [D, H]
                matrix at the end, so the transpose/pool chain never
                waits on the denominator.  Z reaches all 128 partitions
                via ones.T @ (idH * zs).
                """
                e_bf, zsb = e_zsb
                zs = small_pool.tile([c.H, 1], FP32, tag="zs")
                # denominator: sum block accums, subtract the host-computed
                # pad contribution (zcorrn is negated on the host)
                nc.vector.tensor_add(
                    zs[:], zsb[:, 0:1], zcn_sb[:, b:b + 1]
                )
                for blk in range(1, len(c.slot_blocks[b])):
                    nc.vector.tensor_add(zs[:], zs[:], zsb[:, blk:blk + 1])
                zdiag = small_pool.tile([c.H, c.H], FP32, tag="zdiag")
                nc.vector.tensor_scalar_mul(zdiag[:], idHf[:], zs[:])
                chunks = c.slot_chunks[b]
                ncb = len(chunks)
                tpt = hps_pool.tile([P, c.CHM, c.H], FP16, tag="h")
                for cn, (n0, cs) in enumerate(chunks):
                    nc.tensor.matmul(
                        tpt[:, cn, :], e_bf[:, n0:n0 + P], idH[:],
                        is_transpose=True,
                        start=(cn == 0), stop=(cn == ncb - 1),
                    )
                eTt = eT_pool.tile([P, c.CHM, c.H], FP16, tag="eT")
                nc.vector.tensor_copy(eTt[:, :ncb], tpt[:, :ncb])
                zbc = hps_pool.tile([P, c.H], FP32, tag="h")
                nc.tensor.matmul(zbc[:], ones4[:], zdiag[:],
                                 start=True, stop=True)
                rzbc = small_pool.tile([P, c.H], FP32, tag="rzbc")
                nc.vector.reciprocal(rzbc[:], zbc[:])
                plt = hps_pool.tile([P, c.KD, c.H], FP32, tag="h")
                for dc in range(c.KD):
                    for cn, (n0, cs) in enumerate(chunks):
                        nc.tensor.matmul(
                            plt[:, dc, :], xn_tiles[cn][:cs, ts(dc, P)],
                            eTt[:cs, cn, :],
                            start=(cn == 0), stop=(cn == ncb - 1),
                        )
                for hd in range(c.H):
                    nc.vector.tensor_scalar_mul(
                        poolT_sb[:, :, hd, b], plt[:, :, hd],
                        rzbc[:, hd:hd + 1],
                    )

            for rep in range(reps):
                prev = None
                for b in range(c.BL):
                    e_zsb, xn_tiles = emit_scores(b, rep)
                    if prev is not None:
                        emit_tail(prev[0], prev[1], prev[2])
                    prev = (b, e_zsb, xn_tiles)
                emit_tail(prev[0], prev[1], prev[2])
                # out projection for ALL batch columns in one pass
                # (4-col moving operands quarter the instruction count)
                ytp = hps_pool.tile([P, c.KD, c.BL], FP32, tag="h")
                for dblk in range(c.KD):
                    for kc in range(c.KOUT):
                        hd, dc = kc // c.KD, kc % c.KD
                        nc.tensor.matmul(
                            ytp[:, dblk, :],
                            wout_sb[:, kc, ts(dblk, P)],
                            poolT_sb[:, dc, hd, :],
                            start=(kc == 0), stop=(kc == c.KOUT - 1),
                        )
                nc.vector.tensor_add(y_sbT[:], ytp[:], boutT_sb[:])
                nc.sync.dma_start(y_d[:], y_sbT[:])
    return nc


def plan_slots(valid_mask, n_cores, BL):
    """Count-sort batches into (core, slot) so each SPMD batch-slot has a
    tight shared token bound."""
    counts = np.asarray(valid_mask).sum(1)
    order = np.argsort(counts, kind="stable")[::-1]
    slot_npads = []
    for bl in range(BL):
        grp = order[bl * n_cores:(bl + 1) * n_cores]
        mx = int(counts[grp].max())
        slot_npads.append(max(256, mx))
    return order, slot_npads


def make_in_maps(x, valid_mask, W1, b1, W2, b2, Wout, bout, n_cores, cfg):
    """Host-side prep: w2-sort heads' dims, compact valid tokens, fp8
    layouts, shard over batch."""
    c = cfg
    f8 = ml_dtypes.float8_e4m3
    f16 = np.float16
    B, N, D = x.shape
    H, _, F = W1.shape
    HF = H * F

    def q8(a):
        return np.asarray(a, np.float32).astype(f8)

    # per-head permutation: large |w2| dims first
    perm = [np.argsort(-np.abs(np.asarray(W2[h], np.float32)),
                       kind="stable") for h in range(H)]
    W1p = np.stack([np.asarray(W1[h], np.float32)[:, perm[h]]
                    for h in range(H)])          # [H, D, F]
    b1p = np.stack([np.asarray(b1[h], np.float32)[perm[h]]
                    for h in range(H)])          # [H, F]
    w2p = np.stack([np.asarray(W2[h], np.float32)[perm[h]]
                    for h in range(H)])          # [H, F]

    W1f = W1p.transpose(1, 0, 2).reshape(D, HF)  # [D, HF] head-major cols
    w18 = q8(8.0 * W1f)
    w1r = 8.0 * W1f - w18.astype(np.float32)

    def dr_pack_w(wmat):
        # [D, M] -> [P, D//256, 2, M] with d = dc*256 + i*128 + p
        Dw, M = wmat.shape
        return np.ascontiguousarray(
            wmat.reshape(Dw // 256, 2, P, M).transpose(2, 0, 1, 3)
        )

    w18_l = dr_pack_w(w18)
    # rank-0 columns of each head, head-major compact [D, H*P]
    r0cols = np.concatenate(
        [np.arange(4 * h * P, (4 * h + 1) * P) for h in range(H)]
    )
    w1lo_l = dr_pack_w(q8(w1r[:, r0cols]))

    # score-dot stationaries: w28[p, pc, j, hd] = q8(8*w2[f]) one-hot by
    # head, f = (2*pc+j)*128 + p, head = (2*pc+j)//4.  Head axis padded
    # 4->16 for the DR LdWeights pair-step%16 rule.
    w2f8 = q8(8.0 * w2p.reshape(HF))
    w2flo = q8(8.0 * w2p.reshape(HF) - w2f8.astype(np.float32))
    w28_l = np.zeros((P, c.PC, 2, 16), f8)
    w28lo_l = np.zeros((P, c.PC // 2, 2, 16), f8)
    for pc in range(c.PC):
        for j in range(2):
            mc = 2 * pc + j
            fidx = mc * P + np.arange(P)
            w28_l[np.arange(P), pc, j, mc // 4] = w2f8[fidx]
            if pc % 2 == 0:
                w28lo_l[np.arange(P), pc // 2, j, mc // 4] = w2flo[fidx]
    w28_l = np.ascontiguousarray(w28_l)
    w28lo_l = np.ascontiguousarray(w28lo_l)

    b1T_l = np.ascontiguousarray(
        b1p.reshape(HF).reshape(c.MC, P).transpose(1, 0).astype(np.float32)
    )

    # pad-token score per head at the device's exact precision:
    # h_pad = e4m3(tanh(b1)), s_pad8 = sum_f w28[f]*h_pad[f] (+ w28lo on
    # top pairs); e_pad = exp(s_pad8/8)
    hpadq = np.asarray(np.tanh(b1p.reshape(HF)), np.float32).astype(f8)
    hpadf = hpadq.astype(np.float32)
    w28f = w2f8.astype(np.float32)
    w28lof = w2flo.astype(np.float32)
    s_pad8 = np.zeros(H, np.float64)
    for mc in range(c.MC):
        fidx = mc * P + np.arange(P)
        hd = mc // 4
        s_pad8[hd] += (w28f[fidx] * hpadf[fidx]).sum()
        if (mc // 2) % 2 == 0:   # top pair -> w2lo residual applies
            s_pad8[hd] += (w28lof[fidx] * hpadf[fidx]).sum()
    e_pad = np.exp(s_pad8 / 8.0)                 # [H]

    wout_l = np.ascontiguousarray(
        np.asarray(Wout, np.float32).reshape(c.KOUT, P, c.D)
        .transpose(1, 0, 2).astype(f16)
    )
    boutT_l = np.ascontiguousarray(
        np.broadcast_to(
            np.asarray(bout, np.float32).reshape(c.KD, P)
            .transpose(1, 0)[:, :, None],
            (P, c.KD, c.BL),
        ).astype(np.float32)
    )

    order, slot_npads = plan_slots(valid_mask, n_cores, c.BL)
    for bl in range(c.BL):
        assert slot_npads[bl] <= c.slot_npads[bl], (
            f"slot {bl}: cfg bound {c.slot_npads[bl]} < data {slot_npads[bl]}"
        )
    in_maps = []
    for core in range(n_cores):
        xt8_a = np.zeros((c.BL, P, c.KDR, 2, c.N_pad), f8)
        xlo_a = np.zeros((c.BL, P, c.KDR, 2, c.N_pad), f8)
        xc_a = np.zeros((c.BL, c.N_pad, D), f16)
        zcn_a = np.zeros((c.H, c.BL), np.float32)
        for bl in range(c.BL):
            bg = int(order[bl * n_cores + core])
            idx = np.where(valid_mask[bg])[0]
            cnt = len(idx)
            assert cnt <= c.slot_npads[bl]
            xv = np.asarray(x[bg][idx], np.float32)        # [cnt, D]
            x8 = xv.astype(f8)
            xr = xv - x8.astype(np.float32)
            xlo = xr.astype(f8)
            # [cnt, D] -> [P, D//256, 2, cnt]
            xt = x8.T.reshape(c.KDR, 2, P, cnt).transpose(2, 0, 1, 3)
            xl = xlo.T.reshape(c.KDR, 2, P, cnt).transpose(2, 0, 1, 3)
            xt8_a[bl, :, :, :, :cnt] = xt
            xlo_a[bl, :, :, :, :cnt] = xl
            xc_a[bl, :cnt] = xv.astype(f16)
            zcn_a[:, bl] = -(c.slot_npads[bl] - cnt) * e_pad
        in_maps.append({
            "xt8": np.ascontiguousarray(xt8_a),
            "xlo": np.ascontiguousarray(xlo_a),
            "xc": np.ascontiguousarray(xc_a),
            "w18": w18_l, "w1lo": w1lo_l,
            "w28": w28_l, "w28lo": w28lo_l, "b1T": b1T_l,
            "zcorrn": zcn_a,
            "wout": wout_l, "boutT": boutT_l,
        })
    return in_maps


_cached = {}
last_results = None


def kernel(x, valid_mask, W1, b1, W2, b2, Wout, bout, trace=False):
    global last_results
    x, valid_mask, W1, b1, W2, b2, Wout, bout = (
        np.asarray(a)
        for a in (x, valid_mask, W1, b1, W2, b2, Wout, bout)
    )
    B = x.shape[0]
    n_cores = 8
    BL = B // n_cores
    order, slot_npads = plan_slots(valid_mask, n_cores, BL)
    n_pad = max(slot_npads)
    cfg = Cfg(BL=BL, N_pad=n_pad, slot_npads=slot_npads)
    key = (B, n_pad, tuple(slot_npads))
    if key not in _cached:
        nc = bacc.Bacc("TRN2", target_bir_lowering=False, debug=False)
        build_kernel(nc, cfg)
        nc.compile()
        _cached[key] = nc
    in_maps = make_in_maps(x, valid_mask, W1, b1, W2, b2, Wout, bout,
                           n_cores, cfg)
    res = run_bass_kernel_spmd(
        _cached[key], in_maps, core_ids=list(range(n_cores)), trace=trace
    )
    last_results = res
    y = np.empty((B, cfg.D), np.float32)
    for core in range(n_cores):
        yT = np.asarray(res.results[core]["y"], np.float32)  # [P, KD, BL]
        yc = yT.transpose(2, 1, 0).reshape(BL, cfg.D)
        for bl in range(BL):
            y[int(order[bl * n_cores + core])] = yc[bl]
    return y


# revision 59
# speedup vs baseline: 1.0106x; 1.0034x over previous
"""AttentionPool Trainium2 Bass kernel (w2-mass-aware precision, fp8 DR).

Reference computation (per batch b):
    h      = tanh(x @ W1 + b1)          # [N, H*F]
    scores = h @ W2 + b2                # [N, H]   (b2 cancels under softmax)
    scores = where(mask, scores, -1e9)
    w      = softmax(scores, axis=N)
    pooled = w.T @ x                    # [H, D]
    y      = concat_h(pooled) @ Wout + bout

Strategy (vs the 151us 8-resid-pass baseline):
 1. Host-side valid-token compaction (~50% of tokens masked) and batch
    count-sorting into (core, slot), as before.
 2. Each head's F=512 hidden dims are PERMUTED so large-|w2| dims come
    first.  Per head, chunk 0 (128 dims) carries ~72% of sum(w2^2),
    chunk 1 ~20%, chunks 2-3 ~7%.  Score noise scales with the w2^2
    mass of the chunk it enters through, so precision is allocated by
    chunk rank:
      - main x8@w18 fp8 DoubleRow pass: all 16 mc chunks (irreducible)
      - residual passes ONLY on the 4 rank-0 chunks: x-residual on
        d>=256 (xlo@w18) + W-residual on d>=512 (x8@w1lo)
      - score dot h@W2 entirely in fp8 DoubleRow (tanh emits fp8
        directly); top pairs (ranks 0,1) get a w2-residual second pass.
    48 PE-cycles/token vs baseline's 80.  numpy-sim absmax 1.59e-2,
    HW-measured 1.57e-2 (gate 2e-2; baseline measured 1.52e-2).
 3. b1 is applied exactly as a per-partition fp32 bias AP in the tanh
    activation (tanh runs per-mc chunk), replacing the ones-row trick.
    b2 cancels under softmax.  w2 is pre-scaled by 8 (power of two,
    exact) so fp8 quantization stays out of subnormals; the exp
    activation un-scales with scale=1/8.
 4. NO pad mask: pad token columns are all-zero in xt8/xlo, so their
    h8 is exactly q8(tanh(b1)) and their pooling contribution is 0
    (xc rows are zero).  They only inflate the softmax denominator by
    (npad-cnt)*e^(s_pad), which the host computes exactly and the
    device subtracts (zcorrn input).  exp reads the score PSUM
    directly per block (accum per block), killing the DVE mask-add
    and the m16 tensor.
 5. Token blocks of TB=1024 (PSUM [128,1024] tiles, ring of 3).  The
    score dot is software-pipelined two pairs behind the h chains so
    the PE never waits on the tanh of the pair it just produced.
 6. The whole pooling path (e weights, x, Wout, pooled) runs fp16.

Layouts (d = dc*256 + i*128 + p for DoubleRow pairs; f = mc*128 + p
with mc = 4*head + rank after the per-head w2-sort):
  xt8  [BL, P, 4, 2, N_pad] fp8   x compacted, transposed, e4m3
  xlo  [BL, P, 4, 2, N_pad] fp8   q8(x - x8), all d
  xc   [BL, N_pad, D]      fp16   natural x for pooling
  w18  [P, 4, 2, HF] fp8          q8(8*W1f)  (W1f column-permuted)
  w1lo [P, 4, 2, 4*128] fp8       q8(8*W1f - w18), rank-0 cols, head-major
  w28  [P, 8, 2, 16] fp8          q8(8*w2) one-hot-by-head, DR pairs
                                  (head axis padded 4->16: DR LdWeights
                                  needs pair-axis byte-step % 16 == 0)
  w28lo[P, 4, 2, 16] fp8          q8(8*w2 - w28) for top pairs
  b1T  [P, 16] fp32               b1 per (p, mc), tanh bias APs
  zcorrn [H, BL] fp32             -(npad-cnt)*e^(s_pad) denominator fix
  wout [P, 32, D] fp16, boutT [P, 8, BL] fp32, y [P, 8, BL] fp32
"""

import numpy as np
import ml_dtypes

import concourse.bass as bass
import concourse.mybir as mybir
import concourse.tile as tile
from concourse import bacc
from concourse.bass import ts
from concourse.bass_utils import run_bass_kernel_spmd
from concourse.masks import make_identity

FP32 = mybir.dt.float32
F8 = mybir.dt.float8e4
FP16 = mybir.dt.float16
AFT = mybir.ActivationFunctionType
DR = mybir.MatmulPerfMode.DoubleRow

P = 128
TB = 1024


def _blocks(n_pad, tb=TB):
    """Split a slot into token blocks.  Oversize slots split into two
    BALANCED blocks (not 1024+tail): the Act engine's fixed per-tanh
    overhead makes tiny tail blocks Act-bound, stalling the next slot
    on the PSUM ring.  54/46 beats 50/50 (swept): the larger first
    block gives the tanh stream catch-up room at the block boundary."""
    if n_pad <= tb:
        return [(0, n_pad)]
    assert n_pad <= 2 * tb
    b0 = (n_pad * 56 // 100 + 15) // 16 * 16
    return [(0, b0), (b0, n_pad - b0)]


class Cfg:
    def __init__(self, BL=4, N=2048, D=1024, H=4, F=512, N_pad=1152,
                 slot_npads=None):
        self.BL, self.N, self.D, self.H, self.F = BL, N, D, H, F
        self.HF = H * F
        self.N_pad = N_pad
        self.KDR = D // 256          # DoubleRow d-chunks (256 each)
        self.MC = self.HF // P       # h col chunks (16)
        self.PC = self.MC // 2       # score-dot pairs (8)
        self.KD = D // P             # 128-chunks of D
        self.KOUT = (H * D) // P     # contraction chunks of the out proj
        self.slot_npads = list(slot_npads) if slot_npads else [N_pad] * BL
        assert len(self.slot_npads) == BL
        assert max(self.slot_npads) <= N_pad
        self.slot_blocks = [_blocks(np_) for np_ in self.slot_npads]
        self.slot_chunks = []
        for np_ in self.slot_npads:
            ch, n0 = [], 0
            while n0 < np_:
                s = min(P, np_ - n0)
                ch.append((n0, s))
                n0 += s
            self.slot_chunks.append(ch)
        self.CHM = max(12, max(len(ch) for ch in self.slot_chunks))


def build_kernel(nc: bass.Bass, cfg: Cfg, reps: int = 1):
    c = cfg
    xt8_d = nc.dram_tensor("xt8", [c.BL, P, c.KDR, 2, c.N_pad], F8,
                           kind="ExternalInput").ap()
    xlo_d = nc.dram_tensor("xlo", [c.BL, P, c.KDR, 2, c.N_pad], F8,
                           kind="ExternalInput").ap()
    xc_d = nc.dram_tensor("xc", [c.BL, c.N_pad, c.D], FP16,
                          kind="ExternalInput").ap()
    w18_d = nc.dram_tensor("w18", [P, c.KDR, 2, c.HF], F8,
                           kind="ExternalInput").ap()
    w1lo_d = nc.dram_tensor("w1lo", [P, c.KDR, 2, c.H * P], F8,
                            kind="ExternalInput").ap()
    w28_d = nc.dram_tensor("w28", [P, c.PC, 2, 16], F8,
                           kind="ExternalInput").ap()
    w28lo_d = nc.dram_tensor("w28lo", [P, c.PC // 2, 2, 16], F8,
                             kind="ExternalInput").ap()
    b1T_d = nc.dram_tensor("b1T", [P, c.MC], FP32,
                           kind="ExternalInput").ap()
    zcn_d = nc.dram_tensor("zcorrn", [c.H, c.BL], FP32,
                           kind="ExternalInput").ap()
    wout_d = nc.dram_tensor("wout", [P, c.KOUT, c.D], FP16,
                            kind="ExternalInput").ap()
    boutT_d = nc.dram_tensor("boutT", [P, c.KD, c.BL], FP32,
                             kind="ExternalInput").ap()
    y_d = nc.dram_tensor("y", [P, c.KD, c.BL], FP32,
                         kind="ExternalOutput").ap()

    with tile.TileContext(nc) as tc:
        with (
            tc.tile_pool(name="const", bufs=1) as const,
            tc.tile_pool(name="xt8p", bufs=3) as xt8_pool,
            tc.tile_pool(name="xlop", bufs=3) as xlo_pool,
            tc.tile_pool(name="h8p", bufs=12) as h8_pool,
            tc.tile_pool(name="xnp", bufs=20) as xn_pool,
            tc.tile_pool(name="ep", bufs=2) as e_pool,
            tc.tile_pool(name="etp", bufs=2) as eT_pool,
            tc.tile_pool(name="smallp", bufs=4) as small_pool,
            tc.tile_pool(name="hps", bufs=3, space="PSUM") as hps_pool,
            tc.tile_pool(name="scps", bufs=1, space="PSUM") as scps_pool,
        ):
            # ---- constants.  scalar/Act queue carries NO DMAs: the
            # issuing engine is blocked for the whole transfer in this
            # model, and Act is the co-bottleneck (tanh).  w18 streams in
            # pair-column slices ordered by first use (odd pairs first,
            # matching block 0's pc_order); b1T leads for the first tanh.
            # x-data rides the SP queue. ----
            w18_sb = const.tile([P, c.KDR, 2, c.HF], F8)
            w1lo_sb = const.tile([P, c.KDR, 2, c.H * P], F8)
            w28_sb = const.tile([P, c.PC, 2, 16], F8)
            w28lo_sb = const.tile([P, c.PC // 2, 2, 16], F8)
            b1T_sb = const.tile([P, c.MC], FP32)
            zcn_sb = const.tile([c.H, c.BL], FP32)
            for pc in (1, 3):
                cs = ts(pc, 256)
                nc.gpsimd.dma_start(w18_sb[:, :, :, cs], w18_d[:, :, :, cs])
            nc.gpsimd.dma_start(b1T_sb[:], b1T_d)
            for pc in (5, 7):
                cs = ts(pc, 256)
                nc.gpsimd.dma_start(w18_sb[:, :, :, cs], w18_d[:, :, :, cs])
            nc.gpsimd.dma_start(w1lo_sb[:], w1lo_d)
            nc.gpsimd.dma_start(w18_sb[:, :, :, ts(0, 256)],
                                w18_d[:, :, :, ts(0, 256)])
            nc.gpsimd.dma_start(w28_sb[:], w28_d)
            nc.gpsimd.dma_start(w28lo_sb[:], w28lo_d)
            for pc in (2, 4, 6):
                cs = ts(pc, 256)
                nc.gpsimd.dma_start(w18_sb[:, :, :, cs], w18_d[:, :, :, cs])
            nc.gpsimd.dma_start(zcn_sb[:], zcn_d)
            # wout, boutT are DMA'd from inside the first block's emission
            # so they queue behind the startup-critical transfers
            boutT_sb = const.tile([P, c.KD, c.BL], FP32)
            wout_sb = const.tile([P, c.KOUT, c.D], FP16)
            idH = const.tile([c.H, c.H], FP16)
            make_identity(nc, idH[:])
            idHf = const.tile([c.H, c.H], FP32)
            make_identity(nc, idHf[:])
            ones4 = const.tile([c.H, P], FP32)
            nc.vector.memset(ones4[:], 1.0)
            # warm the activation table (Tanh/Exp share one set) during
            # the initial DMA wait
            warm = const.tile([1, 1], FP32)
            nc.scalar.activation(warm[:], idH[:1, :1], AFT.Tanh)
            poolT_sb = const.tile([P, c.KD, c.H, c.BL], FP16)
            y_sbT = const.tile([P, c.KD, c.BL], FP32)

            def emit_xn_dmas(b):
                tiles = []
                for n0, cs in c.slot_chunks[b]:
                    xn = xn_pool.tile([P, c.D], FP16, tag="xn")
                    nc.gpsimd.dma_start(xn[:cs], xc_d[b, n0:n0 + cs, :])
                    tiles.append(xn)
                return tiles

            def emit_scores(b, rep):
                """DMA + h matmuls + tanh(fp8 out) + fp8-DR score dot +
                per-block exp straight off the score PSUM."""
                # +128 columns so a partial tail chunk transposes as a
                # full 128-wide chunk (zeroed here, off the critical path)
                e_bf = e_pool.tile([c.H, c.N_pad + 128], FP16, tag="e")
                zsb = small_pool.tile([c.H, len(c.slot_blocks[b])],
                                      FP32, tag="zsb")
                nbv = c.slot_npads[b]
                padv = (-nbv) % P
                if padv:
                    nc.vector.memset(e_bf[:, nbv:nbv + padv], 0.0)
                first = b == 0 and rep == 0
                for blk, (n0, tb) in enumerate(c.slot_blocks[b]):
                    xt8 = xt8_pool.tile([P, c.KDR, 2, TB], F8, tag="xt8")
                    xlo = xlo_pool.tile([P, c.KDR, 2, TB], F8, tag="xlo")
                    if first and blk == 0:
                        # per-dc split so the first chains start as soon
                        # as their own chunk lands
                        for dc in range(c.KDR):
                            nc.sync.dma_start(
                                xt8[:, dc, :, :tb],
                                xt8_d[b, :, dc, :, n0:n0 + tb],
                            )
                        for dc in range(c.KDR):
                            nc.sync.dma_start(
                                xlo[:, dc, :, :tb],
                                xlo_d[b, :, dc, :, n0:n0 + tb],
                            )
                        nc.gpsimd.dma_start(wout_sb[:], wout_d)
                        nc.gpsimd.dma_start(boutT_sb[:], boutT_d)
                    else:
                        nc.sync.dma_start(
                            xt8[:, :, :, :tb], xt8_d[b, :, :, :, n0:n0 + tb]
                        )
                        nc.sync.dma_start(
                            xlo[:, :, :, :tb], xlo_d[b, :, :, :, n0:n0 + tb]
                        )
                    # first block: residual-free odd pairs first so the
                    # PE isn't gated on the xlo DMA splits at startup
                    if first and blk == 0:
                        pc_order = [1, 3, 5, 7, 0, 2, 4, 6]
                    else:
                        pc_order = list(range(c.PC))
                    h8s = [None] * c.PC
                    hv = [(s0, min(512, tb - s0)) for s0 in range(0, tb, 512)]
                    scp = scps_pool.tile([16, TB], FP32, tag="scps")
                    # each 512-col PSUM half is its own accumulation
                    # group: first/last instr per half carry start/stop
                    _per_half = c.PC + c.PC // 2
                    n_sc = [0] * len(hv)

                    def emit_score(pc):
                        for hi, (s0, sw) in enumerate(hv):
                            sl = slice(s0, s0 + sw)
                            nc.tensor.matmul(
                                scp[:, sl], w28_sb[:, pc],
                                h8s[pc][:, :, sl],
                                start=(n_sc[hi] == 0),
                                stop=(n_sc[hi] == _per_half - 1),
                                perf_mode=DR,
                            )
                            n_sc[hi] += 1
                            if pc % 2 == 0:
                                nc.tensor.matmul(
                                    scp[:, sl], w28lo_sb[:, pc // 2],
                                    h8s[pc][:, :, sl],
                                    start=False,
                                    stop=(n_sc[hi] == _per_half - 1),
                                    perf_mode=DR,
                                )
                                n_sc[hi] += 1
                    pend = []
                    for pc in pc_order:
                        h8 = h8_pool.tile([P, 2, TB], F8, tag="h8")
                        top = pc % 2 == 0   # ranks {0,1} pair of head pc//2
                        hd = pc // 2
                        for j in range(2):
                            mc = 2 * pc + j
                            r0 = top and j == 0  # rank-0 chunk
                            ms = ts(mc, P)
                            hp = hps_pool.tile([P, TB], FP32, tag="h")
                            for s0, sw in hv:
                                sl = slice(s0, s0 + sw)
                                for dc in range(c.KDR):
                                    nc.tensor.matmul(
                                        hp[:, sl],
                                        w18_sb[:, dc, :, ms],
                                        xt8[:, dc, :, sl],
                                        start=(dc == 0),
                                        stop=(dc == c.KDR - 1 and not r0),
                                        perf_mode=DR,
                                    )
                                if r0:
                                    # x-resid on d>=256 + W-resid d>=512
                                    # (the d<256 quarter is dropped: its
                                    # noise share is small vs 2 cyc/tok)
                                    for dc in (1, 2, 3):
                                        nc.tensor.matmul(
                                            hp[:, sl],
                                            w18_sb[:, dc, :, ms],
                                            xlo[:, dc, :, sl],
                                            start=False, stop=False,
                                            perf_mode=DR,
                                        )
                                    for dc in (2, 3):
                                        nc.tensor.matmul(
                                            hp[:, sl],
                                            w1lo_sb[:, dc, :, ts(hd, P)],
                                            xt8[:, dc, :, sl],
                                            start=False, stop=(dc == 3),
                                            perf_mode=DR,
                                        )
                            nc.scalar.activation(
                                h8[:, j, :tb], hp[:, :tb], AFT.Tanh,
                                bias=b1T_sb[:, mc:mc + 1], scale=0.125,
                            )
                        h8s[pc] = h8
                        # score dot runs two pairs behind the chains so
                        # the PE never waits on the tanh it just fed
                        pend.append(pc)
                        lag = 8 if (first and blk > 0) else 4
                        if len(pend) > lag:
                            emit_score(pend.pop(0))
                    for pc in pend:
                        emit_score(pc)
                    # per-block exp straight off the score PSUM (scores
                    # are at 8x scale; pad tokens included, fixed via
                    # zcorrn in the softmax denominator)
                    nc.scalar.activation(
                        e_bf[:, n0:n0 + tb], scp[:c.H, :tb],
                        AFT.Exp, bias=0.0, scale=0.125,
                        accum_out=zsb[:, blk:blk + 1],
                    )
                xn_tiles = emit_xn_dmas(b)
                return (e_bf, zsb), xn_tiles

            def emit_tail(b, e_zsb, xn_tiles):
                """softmax + pooling for batch b (runs under b+1's scores).

                Pooling contracts the RAW (unnormalized) e values; the
                1/Z per-head scale is applied to the tiny pooled [D, H]
                matrix at the end, so the transpose/pool chain never
                waits on the denominator.  Z reaches all 128 partitions
                via ones.T @ (idH * zs).
                """
                e_bf, zsb = e_zsb
                zs = small_pool.tile([c.H, 1], FP32, tag="zs")
                # denominator: sum block accums, subtract the host-computed
                # pad contribution (zcorrn is negated on the host)
                nc.vector.tensor_add(
                    zs[:], zsb[:, 0:1], zcn_sb[:, b:b + 1]
                )
                for blk in range(1, len(c.slot_blocks[b])):
                    nc.vector.tensor_add(zs[:], zs[:], zsb[:, blk:blk + 1])
                zdiag = small_pool.tile([c.H, c.H], FP32, tag="zdiag")
                nc.vector.tensor_scalar_mul(zdiag[:], idHf[:], zs[:])
                chunks = c.slot_chunks[b]
                ncb = len(chunks)
                tpt = hps_pool.tile([P, c.CHM, c.H], FP16, tag="h")
                for cn, (n0, cs) in enumerate(chunks):
                    nc.tensor.matmul(
                        tpt[:, cn, :], e_bf[:, n0:n0 + P], idH[:],
                        is_transpose=True,
                        start=(cn == 0), stop=(cn == ncb - 1),
                    )
                eTt = eT_pool.tile([P, c.CHM, c.H], FP16, tag="eT")
                nc.vector.tensor_copy(eTt[:, :ncb], tpt[:, :ncb])
                zbc = hps_pool.tile([P, c.H], FP32, tag="h")
                nc.tensor.matmul(zbc[:], ones4[:], zdiag[:],
                                 start=True, stop=True)
                rzbc = small_pool.tile([P, c.H], FP32, tag="rzbc")
                nc.vector.reciprocal(rzbc[:], zbc[:])
                plt = hps_pool.tile([P, c.KD, c.H], FP32, tag="h")
                for dc in range(c.KD):
                    for cn, (n0, cs) in enumerate(chunks):
                        nc.tensor.matmul(
                            plt[:, dc, :], xn_tiles[cn][:cs, ts(dc, P)],
                            eTt[:cs, cn, :],
                            start=(cn == 0), stop=(cn == ncb - 1),
                        )
                for hd in range(c.H):
                    nc.vector.tensor_scalar_mul(
                        poolT_sb[:, :, hd, b], plt[:, :, hd],
                        rzbc[:, hd:hd + 1],
                    )

            for rep in range(reps):
                prev = None
                for b in range(c.BL):
                    e_zsb, xn_tiles = emit_scores(b, rep)
                    if prev is not None:
                        emit_tail(prev[0], prev[1], prev[2])
                    prev = (b, e_zsb, xn_tiles)
                emit_tail(prev[0], prev[1], prev[2])
                # out projection for ALL batch columns in one pass
                # (4-col moving operands quarter the instruction count)
                ytp = hps_pool.tile([P, c.KD, c.BL], FP32, tag="h")
                for dblk in range(c.KD):
                    for kc in range(c.KOUT):
                        hd, dc = kc // c.KD, kc % c.KD
                        nc.tensor.matmul(
                            ytp[:, dblk, :],
                            wout_sb[:, kc, ts(dblk, P)],
                            poolT_sb[:, dc, hd, :],
                            start=(kc == 0), stop=(kc == c.KOUT - 1),
                        )
                nc.vector.tensor_add(y_sbT[:], ytp[:], boutT_sb[:])
                nc.sync.dma_start(y_d[:], y_sbT[:])
    return nc


def plan_slots(valid_mask, n_cores, BL):
    """Count-sort batches into (core, slot) so each SPMD batch-slot has a
    tight shared token bound."""
    counts = np.asarray(valid_mask).sum(1)
    order = np.argsort(counts, kind="stable")[::-1]
    slot_npads = []
    for bl in range(BL):
        grp = order[bl * n_cores:(bl + 1) * n_cores]
        mx = int(counts[grp].max())
        slot_npads.append(max(256, mx))
    return order, slot_npads


def make_in_maps(x, valid_mask, W1, b1, W2, b2, Wout, bout, n_cores, cfg):
    """Host-side prep: w2-sort heads' dims, compact valid tokens, fp8
    layouts, shard over batch."""
    c = cfg
    f8 = ml_dtypes.float8_e4m3
    f16 = np.float16
    B, N, D = x.shape
    H, _, F = W1.shape
    HF = H * F

    def q8(a):
        return np.asarray(a, np.float32).astype(f8)

    # per-head permutation: large |w2| dims first
    perm = [np.argsort(-np.abs(np.asarray(W2[h], np.float32)),
                       kind="stable") for h in range(H)]
    W1p = np.stack([np.asarray(W1[h], np.float32)[:, perm[h]]
                    for h in range(H)])          # [H, D, F]
    b1p = np.stack([np.asarray(b1[h], np.float32)[perm[h]]
                    for h in range(H)])          # [H, F]
    w2p = np.stack([np.asarray(W2[h], np.float32)[perm[h]]
                    for h in range(H)])          # [H, F]

    W1f = W1p.transpose(1, 0, 2).reshape(D, HF)  # [D, HF] head-major cols
    w18 = q8(8.0 * W1f)
    w1r = 8.0 * W1f - w18.astype(np.float32)

    def dr_pack_w(wmat):
        # [D, M] -> [P, D//256, 2, M] with d = dc*256 + i*128 + p
        Dw, M = wmat.shape
        return np.ascontiguousarray(
            wmat.reshape(Dw // 256, 2, P, M).transpose(2, 0, 1, 3)
        )

    w18_l = dr_pack_w(w18)
    # rank-0 columns of each head, head-major compact [D, H*P]
    r0cols = np.concatenate(
        [np.arange(4 * h * P, (4 * h + 1) * P) for h in range(H)]
    )
    w1lo_l = dr_pack_w(q8(w1r[:, r0cols]))

    # score-dot stationaries: w28[p, pc, j, hd] = q8(8*w2[f]) one-hot by
    # head, f = (2*pc+j)*128 + p, head = (2*pc+j)//4.  Head axis padded
    # 4->16 for the DR LdWeights pair-step%16 rule.
    w2f8 = q8(8.0 * w2p.reshape(HF))
    w2flo = q8(8.0 * w2p.reshape(HF) - w2f8.astype(np.float32))
    w28_l = np.zeros((P, c.PC, 2, 16), f8)
    w28lo_l = np.zeros((P, c.PC // 2, 2, 16), f8)
    for pc in range(c.PC):
        for j in range(2):
            mc = 2 * pc + j
            fidx = mc * P + np.arange(P)
            w28_l[np.arange(P), pc, j, mc // 4] = w2f8[fidx]
            if pc % 2 == 0:
                w28lo_l[np.arange(P), pc // 2, j, mc // 4] = w2flo[fidx]
    w28_l = np.ascontiguousarray(w28_l)
    w28lo_l = np.ascontiguousarray(w28lo_l)

    b1T_l = np.ascontiguousarray(
        b1p.reshape(HF).reshape(c.MC, P).transpose(1, 0).astype(np.float32)
    )

    # pad-token score per head at the device's exact precision:
    # h_pad = e4m3(tanh(b1)), s_pad8 = sum_f w28[f]*h_pad[f] (+ w28lo on
    # top pairs); e_pad = exp(s_pad8/8)
    hpadq = np.asarray(np.tanh(b1p.reshape(HF)), np.float32).astype(f8)
    hpadf = hpadq.astype(np.float32)
    w28f = w2f8.astype(np.float32)
    w28lof = w2flo.astype(np.float32)
    s_pad8 = np.zeros(H, np.float64)
    for mc in range(c.MC):
        fidx = mc * P + np.arange(P)
        hd = mc // 4
        s_pad8[hd] += (w28f[fidx] * hpadf[fidx]).sum()
        if (mc // 2) % 2 == 0:   # top pair -> w2lo residual applies
            s_pad8[hd] += (w28lof[fidx] * hpadf[fidx]).sum()
    e_pad = np.exp(s_pad8 / 8.0)                 # [H]

    wout_l = np.ascontiguousarray(
        np.asarray(Wout, np.float32).reshape(c.KOUT, P, c.D)
        .transpose(1, 0, 2).astype(f16)
    )
    boutT_l = np.ascontiguousarray(
        np.broadcast_to(
            np.asarray(bout, np.float32).reshape(c.KD, P)
            .transpose(1, 0)[:, :, None],
            (P, c.KD, c.BL),
        ).astype(np.float32)
    )

    order, slot_npads = plan_slots(valid_mask, n_cores, c.BL)
    for bl in range(c.BL):
        assert slot_npads[bl] <= c.slot_npads[bl], (
            f"slot {bl}: cfg bound {c.slot_npads[bl]} < data {slot_npads[bl]}"
        )
    in_maps = []
    for core in range(n_cores):
        xt8_a = np.zeros((c.BL, P, c.KDR, 2, c.N_pad), f8)
        xlo_a = np.zeros((c.BL, P, c.KDR, 2, c.N_pad), f8)
        xc_a = np.zeros((c.BL, c.N_pad, D), f16)
        zcn_a = np.zeros((c.H, c.BL), np.float32)
        for bl in range(c.BL):
            bg = int(order[bl * n_cores + core])
            idx = np.where(valid_mask[bg])[0]
            cnt = len(idx)
            assert cnt <= c.slot_npads[bl]
            xv = np.asarray(x[bg][idx], np.float32)        # [cnt, D]
            x8 = xv.astype(f8)
            xr = xv - x8.astype(np.float32)
            xlo = xr.astype(f8)
            # [cnt, D] -> [P, D//256, 2, cnt]
            xt = x8.T.reshape(c.KDR, 2, P, cnt).transpose(2, 0, 1, 3)
            xl = xlo.T.reshape(c.KDR, 2, P, cnt).transpose(2, 0, 1, 3)
            xt8_a[bl, :, :, :, :cnt] = xt
            xlo_a[bl, :, :, :, :cnt] = xl
            xc_a[bl, :cnt] = xv.astype(f16)
            zcn_a[:, bl] = -(c.slot_npads[bl] - cnt) * e_pad
        in_maps.append({
            "xt8": np.ascontiguousarray(xt8_a),
            "xlo": np.ascontiguousarray(xlo_a),
            "xc": np.ascontiguousarray(xc_a),
            "w18": w18_l, "w1lo": w1lo_l,
            "w28": w28_l, "w28lo": w28lo_l, "b1T": b1T_l,
            "zcorrn": zcn_a,
            "wout": wout_l, "boutT": boutT_l,
        })
    return in_maps


_cached = {}
last_results = None


def kernel(x, valid_mask, W1, b1, W2, b2, Wout, bout, trace=False):
    global last_results
    x, valid_mask, W1, b1, W2, b2, Wout, bout = (
        np.asarray(a)
        for a in (x, valid_mask, W1, b1, W2, b2, Wout, bout)
    )
    B = x.shape[0]
    n_cores = 8
    BL = B // n_cores
    order, slot_npads = plan_slots(valid_mask, n_cores, BL)
    n_pad = max(slot_npads)
    cfg = Cfg(BL=BL, N_pad=n_pad, slot_npads=slot_npads)
    key = (B, n_pad, tuple(slot_npads))
    if key not in _cached:
        nc = bacc.Bacc("TRN2", target_bir_lowering=False, debug=False)
        build_kernel(nc, cfg)
        nc.compile()
        _cached[key] = nc
    in_maps = make_in_maps(x, valid_mask, W1, b1, W2, b2, Wout, bout,
                           n_cores, cfg)
    res = run_bass_kernel_spmd(
        _cached[key], in_maps, core_ids=list(range(n_cores)), trace=trace
    )
    last_results = res
    y = np.empty((B, cfg.D), np.float32)
    for core in range(n_cores):
        yT = np.asarray(res.results[core]["y"], np.float32)  # [P, KD, BL]
        yc = yT.transpose(2, 1, 0).reshape(BL, cfg.D)
        for bl in range(BL):
            y[int(order[bl * n_cores + core])] = yc[bl]
    return y


# revision 61
# speedup vs baseline: 1.0130x; 1.0024x over previous
"""AttentionPool Trainium2 Bass kernel (w2-mass-aware precision, fp8 DR).

Reference computation (per batch b):
    h      = tanh(x @ W1 + b1)          # [N, H*F]
    scores = h @ W2 + b2                # [N, H]   (b2 cancels under softmax)
    scores = where(mask, scores, -1e9)
    w      = softmax(scores, axis=N)
    pooled = w.T @ x                    # [H, D]
    y      = concat_h(pooled) @ Wout + bout

Strategy (vs the 151us 8-resid-pass baseline):
 1. Host-side valid-token compaction (~50% of tokens masked) and batch
    count-sorting into (core, slot), as before.
 2. Each head's F=512 hidden dims are PERMUTED so large-|w2| dims come
    first.  Per head, chunk 0 (128 dims) carries ~72% of sum(w2^2),
    chunk 1 ~20%, chunks 2-3 ~7%.  Score noise scales with the w2^2
    mass of the chunk it enters through, so precision is allocated by
    chunk rank:
      - main x8@w18 fp8 DoubleRow pass: all 16 mc chunks (irreducible)
      - residual passes ONLY on the 4 rank-0 chunks: x-residual on
        d>=256 (xlo@w18) + W-residual on d>=512 (x8@w1lo)
      - score dot h@W2 entirely in fp8 DoubleRow (tanh emits fp8
        directly); top pairs (ranks 0,1) get a w2-residual second pass.
    48 PE-cycles/token vs baseline's 80.  numpy-sim absmax 1.59e-2,
    HW-measured 1.57e-2 (gate 2e-2; baseline measured 1.52e-2).
 3. b1 is applied exactly as a per-partition fp32 bias AP in the tanh
    activation (tanh runs per-mc chunk), replacing the ones-row trick.
    b2 cancels under softmax.  w2 is pre-scaled by 8 (power of two,
    exact) so fp8 quantization stays out of subnormals; the exp
    activation un-scales with scale=1/8.
 4. NO pad mask: pad token columns are all-zero in xt8/xlo, so their
    h8 is exactly q8(tanh(b1)) and their pooling contribution is 0
    (xc rows are zero).  They only inflate the softmax denominator by
    (npad-cnt)*e^(s_pad), which the host computes exactly and the
    device subtracts (zcorrn input).  exp reads the score PSUM
    directly per block (accum per block), killing the DVE mask-add
    and the m16 tensor.
 5. Token blocks of TB=1024 (PSUM [128,1024] tiles, ring of 3).  The
    score dot is software-pipelined two pairs behind the h chains so
    the PE never waits on the tanh of the pair it just produced.
 6. The whole pooling path (e weights, x, Wout, pooled) runs fp16.

Layouts (d = dc*256 + i*128 + p for DoubleRow pairs; f = mc*128 + p
with mc = 4*head + rank after the per-head w2-sort):
  xt8  [BL, P, 4, 2, N_pad] fp8   x compacted, transposed, e4m3
  xlo  [BL, P, 4, 2, N_pad] fp8   q8(x - x8), all d
  xc   [BL, N_pad, D]      fp16   natural x for pooling
  w18  [P, 4, 2, HF] fp8          q8(8*W1f)  (W1f column-permuted)
  w1lo [P, 4, 2, 4*128] fp8       q8(8*W1f - w18), rank-0 cols, head-major
  w28  [P, 8, 2, 16] fp8          q8(8*w2) one-hot-by-head, DR pairs
                                  (head axis padded 4->16: DR LdWeights
                                  needs pair-axis byte-step % 16 == 0)
  w28lo[P, 4, 2, 16] fp8          q8(8*w2 - w28) for top pairs
  b1T  [P, 16] fp32               b1 per (p, mc), tanh bias APs
  zcorrn [H, BL] fp32             -(npad-cnt)*e^(s_pad) denominator fix
  wout [P, 32, D] fp16, boutT [P, 8, BL] fp32, y [P, 8, BL] fp32
"""

import numpy as np
import ml_dtypes

import concourse.bass as bass
import concourse.mybir as mybir
import concourse.tile as tile
from concourse import bacc
from concourse.bass import ts
from concourse.bass_utils import run_bass_kernel_spmd
from concourse.masks import make_identity

FP32 = mybir.dt.float32
F8 = mybir.dt.float8e4
FP16 = mybir.dt.float16
AFT = mybir.ActivationFunctionType
DR = mybir.MatmulPerfMode.DoubleRow

P = 128
TB = 1024


def _blocks(n_pad, tb=TB):
    """Split a slot into token blocks.  Oversize slots split into two
    BALANCED blocks (not 1024+tail): the Act engine's fixed per-tanh
    overhead makes tiny tail blocks Act-bound, stalling the next slot
    on the PSUM ring.  54/46 beats 50/50 (swept): the larger first
    block gives the tanh stream catch-up room at the block boundary."""
    if n_pad <= tb:
        return [(0, n_pad)]
    assert n_pad <= 2 * tb
    b0 = (n_pad * 56 // 100 + 15) // 16 * 16
    return [(0, b0), (b0, n_pad - b0)]


class Cfg:
    def __init__(self, BL=4, N=2048, D=1024, H=4, F=512, N_pad=1152,
                 slot_npads=None):
        self.BL, self.N, self.D, self.H, self.F = BL, N, D, H, F
        self.HF = H * F
        self.N_pad = N_pad
        self.KDR = D // 256          # DoubleRow d-chunks (256 each)
        self.MC = self.HF // P       # h col chunks (16)
        self.PC = self.MC // 2       # score-dot pairs (8)
        self.KD = D // P             # 128-chunks of D
        self.KOUT = (H * D) // P     # contraction chunks of the out proj
        self.slot_npads = list(slot_npads) if slot_npads else [N_pad] * BL
        assert len(self.slot_npads) == BL
        assert max(self.slot_npads) <= N_pad
        self.slot_blocks = [_blocks(np_) for np_ in self.slot_npads]
        self.slot_chunks = []
        for np_ in self.slot_npads:
            ch, n0 = [], 0
            while n0 < np_:
                s = min(P, np_ - n0)
                ch.append((n0, s))
                n0 += s
            self.slot_chunks.append(ch)
        self.CHM = max(12, max(len(ch) for ch in self.slot_chunks))


def build_kernel(nc: bass.Bass, cfg: Cfg, reps: int = 1):
    c = cfg
    xt8_d = nc.dram_tensor("xt8", [c.BL, P, c.KDR, 2, c.N_pad], F8,
                           kind="ExternalInput").ap()
    xlo_d = nc.dram_tensor("xlo", [c.BL, P, c.KDR, 2, c.N_pad], F8,
                           kind="ExternalInput").ap()
    xc_d = nc.dram_tensor("xc", [c.BL, c.N_pad, c.D], FP16,
                          kind="ExternalInput").ap()
    w18_d = nc.dram_tensor("w18", [P, c.KDR, 2, c.HF], F8,
                           kind="ExternalInput").ap()
    w1lo_d = nc.dram_tensor("w1lo", [P, c.KDR, 2, c.H * P], F8,
                            kind="ExternalInput").ap()
    w28_d = nc.dram_tensor("w28", [P, c.PC, 2, 16], F8,
                           kind="ExternalInput").ap()
    w28lo_d = nc.dram_tensor("w28lo", [P, c.PC // 2, 2, 16], F8,
                             kind="ExternalInput").ap()
    b1T_d = nc.dram_tensor("b1T", [P, c.MC], FP32,
                           kind="ExternalInput").ap()
    zcn_d = nc.dram_tensor("zcorrn", [c.H, c.BL], FP32,
                           kind="ExternalInput").ap()
    wout_d = nc.dram_tensor("wout", [P, c.KOUT, c.D], FP16,
                            kind="ExternalInput").ap()
    boutT_d = nc.dram_tensor("boutT", [P, c.KD, c.BL], FP32,
                             kind="ExternalInput").ap()
    y_d = nc.dram_tensor("y", [P, c.KD, c.BL], FP32,
                         kind="ExternalOutput").ap()

    with tile.TileContext(nc) as tc:
        with (
            tc.tile_pool(name="const", bufs=1) as const,
            tc.tile_pool(name="xt8p", bufs=3) as xt8_pool,
            tc.tile_pool(name="xlop", bufs=3) as xlo_pool,
            tc.tile_pool(name="h8p", bufs=12) as h8_pool,
            tc.tile_pool(name="xnp", bufs=20) as xn_pool,
            tc.tile_pool(name="ep", bufs=2) as e_pool,
            tc.tile_pool(name="etp", bufs=2) as eT_pool,
            tc.tile_pool(name="smallp", bufs=4) as small_pool,
            tc.tile_pool(name="hps", bufs=3, space="PSUM") as hps_pool,
            tc.tile_pool(name="scps", bufs=1, space="PSUM") as scps_pool,
        ):
            # ---- constants.  scalar/Act queue carries NO DMAs: the
            # issuing engine is blocked for the whole transfer in this
            # model, and Act is the co-bottleneck (tanh).  w18 streams in
            # pair-column slices ordered by first use (odd pairs first,
            # matching block 0's pc_order); b1T leads for the first tanh.
            # x-data rides the SP queue. ----
            w18_sb = const.tile([P, c.KDR, 2, c.HF], F8)
            w1lo_sb = const.tile([P, c.KDR, 2, c.H * P], F8)
            w28_sb = const.tile([P, c.PC, 2, 16], F8)
            w28lo_sb = const.tile([P, c.PC // 2, 2, 16], F8)
            b1T_sb = const.tile([P, c.MC], FP32)
            zcn_sb = const.tile([c.H, c.BL], FP32)
            for pc in (1, 3):
                cs = ts(pc, 256)
                nc.gpsimd.dma_start(w18_sb[:, :, :, cs], w18_d[:, :, :, cs])
            nc.gpsimd.dma_start(b1T_sb[:], b1T_d)
            for pc in (5, 7):
                cs = ts(pc, 256)
                nc.gpsimd.dma_start(w18_sb[:, :, :, cs], w18_d[:, :, :, cs])
            nc.gpsimd.dma_start(w1lo_sb[:], w1lo_d)
            nc.gpsimd.dma_start(w18_sb[:, :, :, ts(0, 256)],
                                w18_d[:, :, :, ts(0, 256)])
            nc.gpsimd.dma_start(w28_sb[:], w28_d)
            nc.gpsimd.dma_start(w28lo_sb[:], w28lo_d)
            for pc in (2, 4, 6):
                cs = ts(pc, 256)
                nc.gpsimd.dma_start(w18_sb[:, :, :, cs], w18_d[:, :, :, cs])
            nc.gpsimd.dma_start(zcn_sb[:], zcn_d)
            # wout, boutT are DMA'd from inside the first block's emission
            # so they queue behind the startup-critical transfers
            boutT_sb = const.tile([P, c.KD, c.BL], FP32)
            wout_sb = const.tile([P, c.KOUT, c.D], FP16)
            idH = const.tile([c.H, c.H], FP16)
            make_identity(nc, idH[:])
            idHf = const.tile([c.H, c.H], FP32)
            make_identity(nc, idHf[:])
            ones4 = const.tile([c.H, P], FP32)
            nc.vector.memset(ones4[:], 1.0)
            # warm the activation table (Tanh/Exp share one set) during
            # the initial DMA wait
            warm = const.tile([1, 1], FP32)
            nc.scalar.activation(warm[:], idH[:1, :1], AFT.Tanh)
            poolT_sb = const.tile([P, c.KD, c.H, c.BL], FP16)
            y_sbT = const.tile([P, c.KD, c.BL], FP32)

            def emit_xn_dmas(b):
                tiles = []
                for n0, cs in c.slot_chunks[b]:
                    xn = xn_pool.tile([P, c.D], FP16, tag="xn")
                    nc.gpsimd.dma_start(xn[:cs], xc_d[b, n0:n0 + cs, :])
                    tiles.append(xn)
                return tiles

            def emit_scores(b, rep):
                """DMA + h matmuls + tanh(fp8 out) + fp8-DR score dot +
                per-block exp straight off the score PSUM."""
                # +128 columns so a partial tail chunk transposes as a
                # full 128-wide chunk (zeroed here, off the critical path)
                e_bf = e_pool.tile([c.H, c.N_pad + 128], FP16, tag="e")
                zsb = small_pool.tile([c.H, len(c.slot_blocks[b])],
                                      FP32, tag="zsb")
                nbv = c.slot_npads[b]
                padv = (-nbv) % P
                if padv:
                    nc.vector.memset(e_bf[:, nbv:nbv + padv], 0.0)
                first = b == 0 and rep == 0
                for blk, (n0, tb) in enumerate(c.slot_blocks[b]):
                    xt8 = xt8_pool.tile([P, c.KDR, 2, TB], F8, tag="xt8")
                    xlo = xlo_pool.tile([P, c.KDR, 2, TB], F8, tag="xlo")
                    if first and blk == 0:
                        # per-dc split so the first chains start as soon
                        # as their own chunk lands
                        for dc in range(c.KDR):
                            nc.sync.dma_start(
                                xt8[:, dc, :, :tb],
                                xt8_d[b, :, dc, :, n0:n0 + tb],
                            )
                        for dc in range(c.KDR):
                            nc.sync.dma_start(
                                xlo[:, dc, :, :tb],
                                xlo_d[b, :, dc, :, n0:n0 + tb],
                            )
                        nc.gpsimd.dma_start(wout_sb[:], wout_d)
                        nc.gpsimd.dma_start(boutT_sb[:], boutT_d)
                    else:
                        nc.sync.dma_start(
                            xt8[:, :, :, :tb], xt8_d[b, :, :, :, n0:n0 + tb]
                        )
                        nc.sync.dma_start(
                            xlo[:, :, :, :tb], xlo_d[b, :, :, :, n0:n0 + tb]
                        )
                    # first block: residual-free odd pairs first so the
                    # PE isn't gated on the xlo DMA splits at startup
                    if first and blk == 0:
                        pc_order = [1, 3, 5, 7, 0, 2, 4, 6]
                    else:
                        pc_order = list(range(c.PC))
                    h8s = [None] * c.PC
                    hv = [(s0, min(512, tb - s0)) for s0 in range(0, tb, 512)]
                    scp = scps_pool.tile([16, TB], FP32, tag="scps")
                    # each 512-col PSUM half is its own accumulation
                    # group: first/last instr per half carry start/stop
                    _per_half = c.PC + c.PC // 2
                    n_sc = [0] * len(hv)

                    def emit_score(pc):
                        for hi, (s0, sw) in enumerate(hv):
                            sl = slice(s0, s0 + sw)
                            nc.tensor.matmul(
                                scp[:, sl], w28_sb[:, pc],
                                h8s[pc][:, :, sl],
                                start=(n_sc[hi] == 0),
                                stop=(n_sc[hi] == _per_half - 1),
                                perf_mode=DR,
                            )
                            n_sc[hi] += 1
                            if pc % 2 == 0:
                                nc.tensor.matmul(
                                    scp[:, sl], w28lo_sb[:, pc // 2],
                                    h8s[pc][:, :, sl],
                                    start=False,
                                    stop=(n_sc[hi] == _per_half - 1),
                                    perf_mode=DR,
                                )
                                n_sc[hi] += 1
                    pend = []
                    for pc in pc_order:
                        h8 = h8_pool.tile([P, 2, TB], F8, tag="h8")
                        top = pc % 2 == 0   # ranks {0,1} pair of head pc//2
                        hd = pc // 2
                        for j in range(2):
                            mc = 2 * pc + j
                            r0 = top and j == 0  # rank-0 chunk
                            ms = ts(mc, P)
                            hp = hps_pool.tile([P, TB], FP32, tag="h")
                            for s0, sw in hv:
                                sl = slice(s0, s0 + sw)
                                for dc in range(c.KDR):
                                    nc.tensor.matmul(
                                        hp[:, sl],
                                        w18_sb[:, dc, :, ms],
                                        xt8[:, dc, :, sl],
                                        start=(dc == 0),
                                        stop=(dc == c.KDR - 1 and not r0),
                                        perf_mode=DR,
                                    )
                                if r0:
                                    # x-resid on d>=256 + W-resid d>=512
                                    # (the d<256 quarter is dropped: its
                                    # noise share is small vs 2 cyc/tok)
                                    for dc in (1, 2, 3):
                                        nc.tensor.matmul(
                                            hp[:, sl],
                                            w18_sb[:, dc, :, ms],
                                            xlo[:, dc, :, sl],
                                            start=False, stop=False,
                                            perf_mode=DR,
                                        )
                                    for dc in (2, 3):
                                        nc.tensor.matmul(
                                            hp[:, sl],
                                            w1lo_sb[:, dc, :, ts(hd, P)],
                                            xt8[:, dc, :, sl],
                                            start=False, stop=(dc == 3),
                                            perf_mode=DR,
                                        )
                            nc.scalar.activation(
                                h8[:, j, :tb], hp[:, :tb], AFT.Tanh,
                                bias=b1T_sb[:, mc:mc + 1], scale=0.125,
                            )
                        h8s[pc] = h8
                        # score dot runs two pairs behind the chains so
                        # the PE never waits on the tanh it just fed
                        pend.append(pc)
                        lag = 8 if (first and blk > 0) else 4
                        if len(pend) > lag:
                            emit_score(pend.pop(0))
                    for pc in pend:
                        emit_score(pc)
                    # per-block exp straight off the score PSUM (scores
                    # are at 8x scale; pad tokens included, fixed via
                    # zcorrn in the softmax denominator)
                    nc.scalar.activation(
                        e_bf[:, n0:n0 + tb], scp[:c.H, :tb],
                        AFT.Exp, bias=0.0, scale=0.125,
                        accum_out=zsb[:, blk:blk + 1],
                    )
                xn_tiles = emit_xn_dmas(b)
                return (e_bf, zsb), xn_tiles

            def emit_tail(b, e_zsb, xn_tiles):
                """softmax + pooling for batch b (runs under b+1's scores).

                Pooling contracts the RAW (unnormalized) e values; the
                1/Z per-head scale is applied to the tiny pooled [D, H]
                matrix at the end, so the transpose/pool chain never
                waits on the denominator.  Z reaches all 128 partitions
                via ones.T @ (idH * zs).
                """
                e_bf, zsb = e_zsb
                zs = small_pool.tile([c.H, 1], FP32, tag="zs")
                # denominator: sum block accums, subtract the host-computed
                # pad contribution (zcorrn is negated on the host)
                nc.vector.tensor_add(
                    zs[:], zsb[:, 0:1], zcn_sb[:, b:b + 1]
                )
                for blk in range(1, len(c.slot_blocks[b])):
                    nc.vector.tensor_add(zs[:], zs[:], zsb[:, blk:blk + 1])
                zdiag = small_pool.tile([c.H, c.H], FP32, tag="zdiag")
                nc.vector.tensor_scalar_mul(zdiag[:], idHf[:], zs[:])
                chunks = c.slot_chunks[b]
                ncb = len(chunks)
                tpt = hps_pool.tile([P, c.CHM, c.H], FP16, tag="h")
                for cn, (n0, cs) in enumerate(chunks):
                    nc.tensor.matmul(
                        tpt[:, cn, :], e_bf[:, n0:n0 + P], idH[:],
                        is_transpose=True,
                        start=(cn == 0), stop=(cn == ncb - 1),
                    )
                eTt = eT_pool.tile([P, c.CHM, c.H], FP16, tag="eT")
                nc.vector.tensor_copy(eTt[:, :ncb], tpt[:, :ncb])
                zbc = hps_pool.tile([P, c.H], FP32, tag="h")
                nc.tensor.matmul(zbc[:], ones4[:], zdiag[:],
                                 start=True, stop=True)
                rzbc = small_pool.tile([P, c.H], FP32, tag="rzbc")
                nc.vector.reciprocal(rzbc[:], zbc[:])
                plt = hps_pool.tile([P, c.KD, c.H], FP32, tag="h")
                for dc in range(c.KD):
                    for cn, (n0, cs) in enumerate(chunks):
                        nc.tensor.matmul(
                            plt[:, dc, :], xn_tiles[cn][:cs, ts(dc, P)],
                            eTt[:cs, cn, :],
                            start=(cn == 0), stop=(cn == ncb - 1),
                        )
                for hd in range(c.H):
                    nc.vector.tensor_scalar_mul(
                        poolT_sb[:, :, hd, b], plt[:, :, hd],
                        rzbc[:, hd:hd + 1],
                    )

            for rep in range(reps):
                prev = None
                for b in range(c.BL):
                    e_zsb, xn_tiles = emit_scores(b, rep)
                    if prev is not None:
                        emit_tail(prev[0], prev[1], prev[2])
                    prev = (b, e_zsb, xn_tiles)
                # out projection split: batch columns 0..BL-2 are ready
                # after the second-to-last tail and run overlapped; only
                # the last batch's quarter sits in the serial end chain
                bl1 = c.BL - 1
                ytpA = hps_pool.tile([P, c.KD, bl1], FP32, tag="h")
                for dblk in range(c.KD):
                    for kc in range(c.KOUT):
                        hd, dc = kc // c.KD, kc % c.KD
                        nc.tensor.matmul(
                            ytpA[:, dblk, :],
                            wout_sb[:, kc, ts(dblk, P)],
                            poolT_sb[:, dc, hd, 0:bl1],
                            start=(kc == 0), stop=(kc == c.KOUT - 1),
                        )
                nc.vector.tensor_add(
                    y_sbT[:, :, 0:bl1], ytpA[:], boutT_sb[:, :, 0:bl1]
                )
                emit_tail(prev[0], prev[1], prev[2])
                ytpB = hps_pool.tile([P, c.KD, 1], FP32, tag="h")
                for dblk in range(c.KD):
                    for kc in range(c.KOUT):
                        hd, dc = kc // c.KD, kc % c.KD
                        nc.tensor.matmul(
                            ytpB[:, dblk, :],
                            wout_sb[:, kc, ts(dblk, P)],
                            poolT_sb[:, dc, hd, bl1:bl1 + 1],
                            start=(kc == 0), stop=(kc == c.KOUT - 1),
                        )
                nc.vector.tensor_add(
                    y_sbT[:, :, bl1:bl1 + 1], ytpB[:],
                    boutT_sb[:, :, bl1:bl1 + 1]
                )
                nc.sync.dma_start(y_d[:], y_sbT[:])
    return nc


def plan_slots(valid_mask, n_cores, BL):
    """Count-sort batches into (core, slot) so each SPMD batch-slot has a
    tight shared token bound."""
    counts = np.asarray(valid_mask).sum(1)
    order = np.argsort(counts, kind="stable")[::-1]
    slot_npads = []
    for bl in range(BL):
        grp = order[bl * n_cores:(bl + 1) * n_cores]
        mx = int(counts[grp].max())
        slot_npads.append(max(256, mx))
    return order, slot_npads


def make_in_maps(x, valid_mask, W1, b1, W2, b2, Wout, bout, n_cores, cfg):
    """Host-side prep: w2-sort heads' dims, compact valid tokens, fp8
    layouts, shard over batch."""
    c = cfg
    f8 = ml_dtypes.float8_e4m3
    f16 = np.float16
    B, N, D = x.shape
    H, _, F = W1.shape
    HF = H * F

    def q8(a):
        return np.asarray(a, np.float32).astype(f8)

    # per-head permutation: large |w2| dims first
    perm = [np.argsort(-np.abs(np.asarray(W2[h], np.float32)),
                       kind="stable") for h in range(H)]
    W1p = np.stack([np.asarray(W1[h], np.float32)[:, perm[h]]
                    for h in range(H)])          # [H, D, F]
    b1p = np.stack([np.asarray(b1[h], np.float32)[perm[h]]
                    for h in range(H)])          # [H, F]
    w2p = np.stack([np.asarray(W2[h], np.float32)[perm[h]]
                    for h in range(H)])          # [H, F]

    W1f = W1p.transpose(1, 0, 2).reshape(D, HF)  # [D, HF] head-major cols
    w18 = q8(8.0 * W1f)
    w1r = 8.0 * W1f - w18.astype(np.float32)

    def dr_pack_w(wmat):
        # [D, M] -> [P, D//256, 2, M] with d = dc*256 + i*128 + p
        Dw, M = wmat.shape
        return np.ascontiguousarray(
            wmat.reshape(Dw // 256, 2, P, M).transpose(2, 0, 1, 3)
        )

    w18_l = dr_pack_w(w18)
    # rank-0 columns of each head, head-major compact [D, H*P]
    r0cols = np.concatenate(
        [np.arange(4 * h * P, (4 * h + 1) * P) for h in range(H)]
    )
    w1lo_l = dr_pack_w(q8(w1r[:, r0cols]))

    # score-dot stationaries: w28[p, pc, j, hd] = q8(8*w2[f]) one-hot by
    # head, f = (2*pc+j)*128 + p, head = (2*pc+j)//4.  Head axis padded
    # 4->16 for the DR LdWeights pair-step%16 rule.
    w2f8 = q8(8.0 * w2p.reshape(HF))
    w2flo = q8(8.0 * w2p.reshape(HF) - w2f8.astype(np.float32))
    w28_l = np.zeros((P, c.PC, 2, 16), f8)
    w28lo_l = np.zeros((P, c.PC // 2, 2, 16), f8)
    for pc in range(c.PC):
        for j in range(2):
            mc = 2 * pc + j
            fidx = mc * P + np.arange(P)
            w28_l[np.arange(P), pc, j, mc // 4] = w2f8[fidx]
            if pc % 2 == 0:
                w28lo_l[np.arange(P), pc // 2, j, mc // 4] = w2flo[fidx]
    w28_l = np.ascontiguousarray(w28_l)
    w28lo_l = np.ascontiguousarray(w28lo_l)

    b1T_l = np.ascontiguousarray(
        b1p.reshape(HF).reshape(c.MC, P).transpose(1, 0).astype(np.float32)
    )

    # pad-token score per head at the device's exact precision:
    # h_pad = e4m3(tanh(b1)), s_pad8 = sum_f w28[f]*h_pad[f] (+ w28lo on
    # top pairs); e_pad = exp(s_pad8/8)
    hpadq = np.asarray(np.tanh(b1p.reshape(HF)), np.float32).astype(f8)
    hpadf = hpadq.astype(np.float32)
    w28f = w2f8.astype(np.float32)
    w28lof = w2flo.astype(np.float32)
    s_pad8 = np.zeros(H, np.float64)
    for mc in range(c.MC):
        fidx = mc * P + np.arange(P)
        hd = mc // 4
        s_pad8[hd] += (w28f[fidx] * hpadf[fidx]).sum()
        if (mc // 2) % 2 == 0:   # top pair -> w2lo residual applies
            s_pad8[hd] += (w28lof[fidx] * hpadf[fidx]).sum()
    e_pad = np.exp(s_pad8 / 8.0)                 # [H]

    wout_l = np.ascontiguousarray(
        np.asarray(Wout, np.float32).reshape(c.KOUT, P, c.D)
        .transpose(1, 0, 2).astype(f16)
    )
    boutT_l = np.ascontiguousarray(
        np.broadcast_to(
            np.asarray(bout, np.float32).reshape(c.KD, P)
            .transpose(1, 0)[:, :, None],
            (P, c.KD, c.BL),
        ).astype(np.float32)
    )

    order, slot_npads = plan_slots(valid_mask, n_cores, c.BL)
    for bl in range(c.BL):
        assert slot_npads[bl] <= c.slot_npads[bl], (
            f"slot {bl}: cfg bound {c.slot_npads[bl]} < data {slot_npads[bl]}"
        )
    in_maps = []
    for core in range(n_cores):
        xt8_a = np.zeros((c.BL, P, c.KDR, 2, c.N_pad), f8)
        xlo_a = np.zeros((c.BL, P, c.KDR, 2, c.N_pad), f8)
        xc_a = np.zeros((c.BL, c.N_pad, D), f16)
        zcn_a = np.zeros((c.H, c.BL), np.float32)
        for bl in range(c.BL):
            bg = int(order[bl * n_cores + core])
            idx = np.where(valid_mask[bg])[0]
            cnt = len(idx)
            assert cnt <= c.slot_npads[bl]
            xv = np.asarray(x[bg][idx], np.float32)        # [cnt, D]
            x8 = xv.astype(f8)
            xr = xv - x8.astype(np.float32)
            xlo = xr.astype(f8)
            # [cnt, D] -> [P, D//256, 2, cnt]
            xt = x8.T.reshape(c.KDR, 2, P, cnt).transpose(2, 0, 1, 3)
            xl = xlo.T.reshape(c.KDR, 2, P, cnt).transpose(2, 0, 1, 3)
            xt8_a[bl, :, :, :, :cnt] = xt
            xlo_a[bl, :, :, :, :cnt] = xl
            xc_a[bl, :cnt] = xv.astype(f16)
            zcn_a[:, bl] = -(c.slot_npads[bl] - cnt) * e_pad
        in_maps.append({
            "xt8": np.ascontiguousarray(xt8_a),
            "xlo": np.ascontiguousarray(xlo_a),
            "xc": np.ascontiguousarray(xc_a),
            "w18": w18_l, "w1lo": w1lo_l,
            "w28": w28_l, "w28lo": w28lo_l, "b1T": b1T_l,
            "zcorrn": zcn_a,
            "wout": wout_l, "boutT": boutT_l,
        })
    return in_maps


_cached = {}
last_results = None


def kernel(x, valid_mask, W1, b1, W2, b2, Wout, bout, trace=False):
    global last_results
    x, valid_mask, W1, b1, W2, b2, Wout, bout = (
        np.asarray(a)
        for a in (x, valid_mask, W1, b1, W2, b2, Wout, bout)
    )
    B = x.shape[0]
    n_cores = 8
    BL = B // n_cores
    order, slot_npads = plan_slots(valid_mask, n_cores, BL)
    n_pad = max(slot_npads)
    cfg = Cfg(BL=BL, N_pad=n_pad, slot_npads=slot_npads)
    key = (B, n_pad, tuple(slot_npads))
    if key not in _cached:
        nc = bacc.Bacc("TRN2", target_bir_lowering=False, debug=False)
        build_kernel(nc, cfg)
        nc.compile()
        _cached[key] = nc
    in_maps = make_in_maps(x, valid_mask, W1, b1, W2, b2, Wout, bout,
                           n_cores, cfg)
    res = run_bass_kernel_spmd(
        _cached[key], in_maps, core_ids=list(range(n_cores)), trace=trace
    )
    last_results = res
    y = np.empty((B, cfg.D), np.float32)
    for core in range(n_cores):
        yT = np.asarray(res.results[core]["y"], np.float32)  # [P, KD, BL]
        yc = yT.transpose(2, 1, 0).reshape(BL, cfg.D)
        for bl in range(BL):
            y[int(order[bl * n_cores + core])] = yc[bl]
    return y


# revision 65
# speedup vs baseline: 1.0184x; 1.0053x over previous
"""AttentionPool Trainium2 Bass kernel (w2-mass-aware precision, fp8 DR).

Reference computation (per batch b):
    h      = tanh(x @ W1 + b1)          # [N, H*F]
    scores = h @ W2 + b2                # [N, H]   (b2 cancels under softmax)
    scores = where(mask, scores, -1e9)
    w      = softmax(scores, axis=N)
    pooled = w.T @ x                    # [H, D]
    y      = concat_h(pooled) @ Wout + bout

Strategy (vs the 151us 8-resid-pass baseline):
 1. Host-side valid-token compaction (~50% of tokens masked) and batch
    count-sorting into (core, slot), as before.
 2. Each head's F=512 hidden dims are PERMUTED so large-|w2| dims come
    first.  Per head, chunk 0 (128 dims) carries ~72% of sum(w2^2),
    chunk 1 ~20%, chunks 2-3 ~7%.  Score noise scales with the w2^2
    mass of the chunk it enters through, so precision is allocated by
    chunk rank:
      - main x8@w18 fp8 DoubleRow pass: all 16 mc chunks (irreducible)
      - residual passes ONLY on the 4 rank-0 chunks: x-residual on
        d>=256 (xlo@w18) + W-residual on d>=512 (x8@w1lo)
      - score dot h@W2 entirely in fp8 DoubleRow (tanh emits fp8
        directly); top pairs (ranks 0,1) get a w2-residual second pass.
    48 PE-cycles/token vs baseline's 80.  numpy-sim absmax 1.59e-2,
    HW-measured 1.57e-2 (gate 2e-2; baseline measured 1.52e-2).
 3. b1 is applied exactly as a per-partition fp32 bias AP in the tanh
    activation (tanh runs per-mc chunk), replacing the ones-row trick.
    b2 cancels under softmax.  w2 is pre-scaled by 8 (power of two,
    exact) so fp8 quantization stays out of subnormals; the exp
    activation un-scales with scale=1/8.
 4. NO pad mask: pad token columns are all-zero in xt8/xlo, so their
    h8 is exactly q8(tanh(b1)) and their pooling contribution is 0
    (xc rows are zero).  They only inflate the softmax denominator by
    (npad-cnt)*e^(s_pad), which the host computes exactly and the
    device subtracts (zcorrn input).  exp reads the score PSUM
    directly per block (accum per block), killing the DVE mask-add
    and the m16 tensor.
 5. Token blocks of TB=1024 (PSUM [128,1024] tiles, ring of 3).  The
    score dot is software-pipelined two pairs behind the h chains so
    the PE never waits on the tanh of the pair it just produced.
 6. The whole pooling path (e weights, x, Wout, pooled) runs fp16.

Layouts (d = dc*256 + i*128 + p for DoubleRow pairs; f = mc*128 + p
with mc = 4*head + rank after the per-head w2-sort):
  xt8  [BL, P, 4, 2, N_pad] fp8   x compacted, transposed, e4m3
  xlo  [BL, P, 4, 2, N_pad] fp8   q8(x - x8), all d
  xc   [BL, N_pad, D]      fp16   natural x for pooling
  w18  [P, 4, 2, HF] fp8          q8(8*W1f)  (W1f column-permuted)
  w1lo [P, 4, 2, 4*128] fp8       q8(8*W1f - w18), rank-0 cols, head-major
  w28  [P, 8, 2, 16] fp8          q8(8*w2) one-hot-by-head, DR pairs
                                  (head axis padded 4->16: DR LdWeights
                                  needs pair-axis byte-step % 16 == 0)
  w28lo[P, 4, 2, 16] fp8          q8(8*w2 - w28) for top pairs
  b1T  [P, 16] fp32               b1 per (p, mc), tanh bias APs
  zcorrn [H, BL] fp32             -(npad-cnt)*e^(s_pad) denominator fix
  wout [P, 32, D] fp16, boutT [P, 8, BL] fp32, y [P, 8, BL] fp32
"""

import numpy as np
import ml_dtypes

import concourse.bass as bass
import concourse.mybir as mybir
import concourse.tile as tile
from concourse import bacc
from concourse.bass import ts
from concourse.bass_utils import run_bass_kernel_spmd
from concourse.masks import make_identity

FP32 = mybir.dt.float32
F8 = mybir.dt.float8e4
FP16 = mybir.dt.float16
AFT = mybir.ActivationFunctionType
DR = mybir.MatmulPerfMode.DoubleRow

P = 128
TB = 1024


def _blocks(n_pad, tb=TB):
    """Split a slot into token blocks.  Oversize slots split into two
    BALANCED blocks (not 1024+tail): the Act engine's fixed per-tanh
    overhead makes tiny tail blocks Act-bound, stalling the next slot
    on the PSUM ring.  54/46 beats 50/50 (swept): the larger first
    block gives the tanh stream catch-up room at the block boundary."""
    if n_pad <= tb:
        return [(0, n_pad)]
    assert n_pad <= 2 * tb
    b0 = (n_pad * 56 // 100 + 15) // 16 * 16
    return [(0, b0), (b0, n_pad - b0)]


class Cfg:
    def __init__(self, BL=4, N=2048, D=1024, H=4, F=512, N_pad=1152,
                 slot_npads=None):
        self.BL, self.N, self.D, self.H, self.F = BL, N, D, H, F
        self.HF = H * F
        self.N_pad = N_pad
        self.KDR = D // 256          # DoubleRow d-chunks (256 each)
        self.MC = self.HF // P       # h col chunks (16)
        self.PC = self.MC // 2       # score-dot pairs (8)
        self.KD = D // P             # 128-chunks of D
        self.KOUT = (H * D) // P     # contraction chunks of the out proj
        self.slot_npads = list(slot_npads) if slot_npads else [N_pad] * BL
        assert len(self.slot_npads) == BL
        assert max(self.slot_npads) <= N_pad
        self.slot_blocks = [_blocks(np_) for np_ in self.slot_npads]
        self.slot_chunks = []
        for np_ in self.slot_npads:
            ch, n0 = [], 0
            while n0 < np_:
                s = min(P, np_ - n0)
                ch.append((n0, s))
                n0 += s
            self.slot_chunks.append(ch)
        self.CHM = max(12, max(len(ch) for ch in self.slot_chunks))


def build_kernel(nc: bass.Bass, cfg: Cfg, reps: int = 1):
    c = cfg
    xt8_d = nc.dram_tensor("xt8", [c.BL, P, c.KDR, 2, c.N_pad], F8,
                           kind="ExternalInput").ap()
    xlo_d = nc.dram_tensor("xlo", [c.BL, P, c.KDR, 2, c.N_pad], F8,
                           kind="ExternalInput").ap()
    xc_d = nc.dram_tensor("xc", [c.BL, c.N_pad, c.D], FP16,
                          kind="ExternalInput").ap()
    w18_d = nc.dram_tensor("w18", [P, c.KDR, 2, c.HF], F8,
                           kind="ExternalInput").ap()
    w1lo_d = nc.dram_tensor("w1lo", [P, c.KDR, 2, c.H * P], F8,
                            kind="ExternalInput").ap()
    w28_d = nc.dram_tensor("w28", [P, c.PC, 2, 16], F8,
                           kind="ExternalInput").ap()
    w28lo_d = nc.dram_tensor("w28lo", [P, c.PC // 2, 2, 16], F8,
                             kind="ExternalInput").ap()
    b1T_d = nc.dram_tensor("b1T", [P, c.MC], FP32,
                           kind="ExternalInput").ap()
    zcn_d = nc.dram_tensor("zcorrn", [c.H, c.BL], FP32,
                           kind="ExternalInput").ap()
    wout_d = nc.dram_tensor("wout", [P, c.KOUT, c.D], FP16,
                            kind="ExternalInput").ap()
    boutT_d = nc.dram_tensor("boutT", [P, c.KD, c.BL], FP32,
                             kind="ExternalInput").ap()
    y_d = nc.dram_tensor("y", [P, c.KD, c.BL], FP32,
                         kind="ExternalOutput").ap()

    with tile.TileContext(nc) as tc:
        with (
            tc.tile_pool(name="const", bufs=1) as const,
            tc.tile_pool(name="xt8p", bufs=3) as xt8_pool,
            tc.tile_pool(name="xlop", bufs=3) as xlo_pool,
            tc.tile_pool(name="h8p", bufs=12) as h8_pool,
            tc.tile_pool(name="xnp", bufs=20) as xn_pool,
            tc.tile_pool(name="ep", bufs=2) as e_pool,
            tc.tile_pool(name="etp", bufs=2) as eT_pool,
            tc.tile_pool(name="smallp", bufs=4) as small_pool,
            tc.tile_pool(name="hps", bufs=3, space="PSUM") as hps_pool,
            tc.tile_pool(name="scps", bufs=1, space="PSUM") as scps_pool,
        ):
            # ---- constants.  scalar/Act queue carries NO DMAs: the
            # issuing engine is blocked for the whole transfer in this
            # model, and Act is the co-bottleneck (tanh).  w18 streams in
            # pair-column slices ordered by first use (odd pairs first,
            # matching block 0's pc_order); b1T leads for the first tanh.
            # x-data rides the SP queue. ----
            w18_sb = const.tile([P, c.KDR, 2, c.HF], F8)
            w1lo_sb = const.tile([P, c.KDR, 2, c.H * P], F8)
            w28_sb = const.tile([P, c.PC, 2, 16], F8)
            w28lo_sb = const.tile([P, c.PC // 2, 2, 16], F8)
            b1T_sb = const.tile([P, c.MC], FP32)
            zcn_sb = const.tile([c.H, c.BL], FP32)
            for pc in (1, 3):
                cs = ts(pc, 256)
                nc.gpsimd.dma_start(w18_sb[:, :, :, cs], w18_d[:, :, :, cs])
            nc.gpsimd.dma_start(b1T_sb[:], b1T_d)
            for pc in (5, 7):
                cs = ts(pc, 256)
                nc.gpsimd.dma_start(w18_sb[:, :, :, cs], w18_d[:, :, :, cs])
            nc.gpsimd.dma_start(w1lo_sb[:], w1lo_d)
            nc.gpsimd.dma_start(w18_sb[:, :, :, ts(0, 256)],
                                w18_d[:, :, :, ts(0, 256)])
            nc.gpsimd.dma_start(w28_sb[:], w28_d)
            nc.gpsimd.dma_start(w28lo_sb[:], w28lo_d)
            for pc in (2, 4, 6):
                cs = ts(pc, 256)
                nc.gpsimd.dma_start(w18_sb[:, :, :, cs], w18_d[:, :, :, cs])
            nc.gpsimd.dma_start(zcn_sb[:], zcn_d)
            # wout, boutT are DMA'd from inside the first block's emission
            # so they queue behind the startup-critical transfers
            boutT_sb = const.tile([P, c.KD, c.BL], FP32)
            wout_sb = const.tile([P, c.KOUT, c.D], FP16)
            idH = const.tile([c.H, c.H], FP16)
            make_identity(nc, idH[:])
            idHf = const.tile([c.H, c.H], FP32)
            make_identity(nc, idHf[:])
            ones4 = const.tile([c.H, P], FP32)
            nc.vector.memset(ones4[:], 1.0)
            # warm the activation table (Tanh/Exp share one set) during
            # the initial DMA wait
            warm = const.tile([1, 1], FP32)
            nc.scalar.activation(warm[:], idH[:1, :1], AFT.Tanh)
            poolT_sb = const.tile([P, c.KD, c.H, c.BL], FP16)
            y_sbT = const.tile([P, c.KD, c.BL], FP32)

            def emit_xn_dmas(b):
                tiles = []
                for n0, cs in c.slot_chunks[b]:
                    xn = xn_pool.tile([P, c.D], FP16, tag="xn")
                    nc.gpsimd.dma_start(xn[:cs], xc_d[b, n0:n0 + cs, :])
                    tiles.append(xn)
                return tiles

            def emit_scores(b, rep):
                """DMA + h matmuls + tanh(fp8 out) + fp8-DR score dot +
                per-block exp straight off the score PSUM."""
                # +128 columns so a partial tail chunk transposes as a
                # full 128-wide chunk (zeroed here, off the critical path)
                e_bf = e_pool.tile([c.H, c.N_pad + 128], FP16, tag="e")
                zsb = small_pool.tile([c.H, len(c.slot_blocks[b])],
                                      FP32, tag="zsb")
                nbv = c.slot_npads[b]
                padv = (-nbv) % P
                if padv:
                    nc.vector.memset(e_bf[:, nbv:nbv + padv], 0.0)
                first = b == 0 and rep == 0
                for blk, (n0, tb) in enumerate(c.slot_blocks[b]):
                    xt8 = xt8_pool.tile([P, c.KDR, 2, TB], F8, tag="xt8")
                    xlo = xlo_pool.tile([P, c.KDR, 2, TB], F8, tag="xlo")
                    if first and blk == 0:
                        # per-dc split so the first chains start as soon
                        # as their own chunk lands
                        for dc in range(c.KDR):
                            nc.sync.dma_start(
                                xt8[:, dc, :, :tb],
                                xt8_d[b, :, dc, :, n0:n0 + tb],
                            )
                        for dc in range(c.KDR):
                            nc.sync.dma_start(
                                xlo[:, dc, :, :tb],
                                xlo_d[b, :, dc, :, n0:n0 + tb],
                            )
                        nc.gpsimd.dma_start(wout_sb[:], wout_d)
                        nc.gpsimd.dma_start(boutT_sb[:], boutT_d)
                    else:
                        nc.sync.dma_start(
                            xt8[:, :, :, :tb], xt8_d[b, :, :, :, n0:n0 + tb]
                        )
                        nc.sync.dma_start(
                            xlo[:, :, :, :tb], xlo_d[b, :, :, :, n0:n0 + tb]
                        )
                    # first block: residual-free odd pairs first so the
                    # PE isn't gated on the xlo DMA splits at startup
                    if first and blk == 0:
                        pc_order = [1, 3, 5, 7, 0, 2, 4, 6]
                    else:
                        pc_order = list(range(c.PC))
                    h8s = [None] * c.PC
                    hv = [(s0, min(512, tb - s0)) for s0 in range(0, tb, 512)]
                    scp = scps_pool.tile([16, TB], FP32, tag="scps")
                    # each 512-col PSUM half is its own accumulation
                    # group: first/last instr per half carry start/stop
                    _per_half = c.PC + c.PC // 2
                    n_sc = [0] * len(hv)

                    def emit_score(pc):
                        for hi, (s0, sw) in enumerate(hv):
                            sl = slice(s0, s0 + sw)
                            nc.tensor.matmul(
                                scp[:, sl], w28_sb[:, pc],
                                h8s[pc][:, :, sl],
                                start=(n_sc[hi] == 0),
                                stop=(n_sc[hi] == _per_half - 1),
                                perf_mode=DR,
                            )
                            n_sc[hi] += 1
                            if pc % 2 == 0:
                                nc.tensor.matmul(
                                    scp[:, sl], w28lo_sb[:, pc // 2],
                                    h8s[pc][:, :, sl],
                                    start=False,
                                    stop=(n_sc[hi] == _per_half - 1),
                                    perf_mode=DR,
                                )
                                n_sc[hi] += 1
                    pend = []
                    for pc in pc_order:
                        h8 = h8_pool.tile([P, 2, TB], F8, tag="h8")
                        top = pc % 2 == 0   # ranks {0,1} pair of head pc//2
                        hd = pc // 2
                        for j in range(2):
                            mc = 2 * pc + j
                            r0 = top and j == 0  # rank-0 chunk
                            ms = ts(mc, P)
                            hp = hps_pool.tile([P, TB], FP32, tag="h")
                            for s0, sw in hv:
                                sl = slice(s0, s0 + sw)
                                for dc in range(c.KDR):
                                    nc.tensor.matmul(
                                        hp[:, sl],
                                        w18_sb[:, dc, :, ms],
                                        xt8[:, dc, :, sl],
                                        start=(dc == 0),
                                        stop=(dc == c.KDR - 1 and not r0),
                                        perf_mode=DR,
                                    )
                                if r0:
                                    # x-resid on d>=256 + W-resid d>=512
                                    # (the d<256 quarter is dropped: its
                                    # noise share is small vs 2 cyc/tok)
                                    for dc in (1, 2, 3):
                                        nc.tensor.matmul(
                                            hp[:, sl],
                                            w18_sb[:, dc, :, ms],
                                            xlo[:, dc, :, sl],
                                            start=False, stop=False,
                                            perf_mode=DR,
                                        )
                                    for dc in (2, 3):
                                        nc.tensor.matmul(
                                            hp[:, sl],
                                            w1lo_sb[:, dc, :, ts(hd, P)],
                                            xt8[:, dc, :, sl],
                                            start=False, stop=(dc == 3),
                                            perf_mode=DR,
                                        )
                            nc.scalar.activation(
                                h8[:, j, :tb], hp[:, :tb], AFT.Tanh,
                                bias=b1T_sb[:, mc:mc + 1], scale=0.125,
                            )
                        h8s[pc] = h8
                        # score dot runs two pairs behind the chains so
                        # the PE never waits on the tanh it just fed
                        pend.append(pc)
                        lag = 8 if (first and blk > 0) else 4
                        if len(pend) > lag:
                            emit_score(pend.pop(0))
                    for pc in pend:
                        emit_score(pc)
                    # per-block exp straight off the score PSUM (scores
                    # are at 8x scale; pad tokens included, fixed via
                    # zcorrn in the softmax denominator)
                    nc.scalar.activation(
                        e_bf[:, n0:n0 + tb], scp[:c.H, :tb],
                        AFT.Exp, bias=0.0, scale=0.125,
                        accum_out=zsb[:, blk:blk + 1],
                    )
                xn_tiles = emit_xn_dmas(b)
                return (e_bf, zsb), xn_tiles

            def emit_tail(b, e_zsb, xn_tiles):
                """softmax + pooling for batch b (runs under b+1's scores).

                Pooling contracts the RAW (unnormalized) e values; the
                1/Z per-head scale is applied to the tiny pooled [D, H]
                matrix at the end, so the transpose/pool chain never
                waits on the denominator.  Z reaches all 128 partitions
                via ones.T @ (idH * zs).
                """
                e_bf, zsb = e_zsb
                zs = small_pool.tile([c.H, 1], FP32, tag="zs")
                # denominator: sum block accums, subtract the host-computed
                # pad contribution (zcorrn is negated on the host)
                nc.vector.tensor_add(
                    zs[:], zsb[:, 0:1], zcn_sb[:, b:b + 1]
                )
                for blk in range(1, len(c.slot_blocks[b])):
                    nc.vector.tensor_add(zs[:], zs[:], zsb[:, blk:blk + 1])
                zdiag = small_pool.tile([c.H, c.H], FP32, tag="zdiag")
                nc.vector.tensor_scalar_mul(zdiag[:], idHf[:], zs[:])
                chunks = c.slot_chunks[b]
                ncb = len(chunks)
                tpt = hps_pool.tile([P, c.CHM, c.H], FP16, tag="h")
                for cn, (n0, cs) in enumerate(chunks):
                    nc.tensor.matmul(
                        tpt[:, cn, :], e_bf[:, n0:n0 + P], idH[:],
                        is_transpose=True,
                        start=(cn == 0), stop=(cn == ncb - 1),
                    )
                eTt = eT_pool.tile([P, c.CHM, c.H], FP16, tag="eT")
                nc.vector.tensor_copy(eTt[:, :ncb], tpt[:, :ncb])
                zbc = hps_pool.tile([P, c.H], FP32, tag="h")
                nc.tensor.matmul(zbc[:], ones4[:], zdiag[:],
                                 start=True, stop=True)
                rzbc = small_pool.tile([P, 1, c.H], FP32, tag="rzbc")
                nc.vector.reciprocal(rzbc[:, 0], zbc[:])
                plt = hps_pool.tile([P, c.KD, c.H], FP32, tag="h")
                for dc in range(c.KD):
                    for cn, (n0, cs) in enumerate(chunks):
                        nc.tensor.matmul(
                            plt[:, dc, :], xn_tiles[cn][:cs, ts(dc, P)],
                            eTt[:cs, cn, :],
                            start=(cn == 0), stop=(cn == ncb - 1),
                        )
                nc.vector.tensor_mul(
                    poolT_sb[:, :, :, b], plt[:],
                    rzbc[:].broadcast_to([P, c.KD, c.H]),
                )

            for rep in range(reps):
                prev = None
                for b in range(c.BL):
                    e_zsb, xn_tiles = emit_scores(b, rep)
                    if prev is not None:
                        emit_tail(prev[0], prev[1], prev[2])
                    prev = (b, e_zsb, xn_tiles)
                # out projection split: batch columns 0..BL-2 are ready
                # after the second-to-last tail and run overlapped; only
                # the last batch's quarter sits in the serial end chain
                bl1 = c.BL - 1
                ytpA = hps_pool.tile([P, c.KD, bl1], FP32, tag="h")
                for dblk in range(c.KD):
                    for kc in range(c.KOUT):
                        hd, dc = kc // c.KD, kc % c.KD
                        nc.tensor.matmul(
                            ytpA[:, dblk, :],
                            wout_sb[:, kc, ts(dblk, P)],
                            poolT_sb[:, dc, hd, 0:bl1],
                            start=(kc == 0), stop=(kc == c.KOUT - 1),
                        )
                nc.vector.tensor_add(
                    y_sbT[:, :, 0:bl1], ytpA[:], boutT_sb[:, :, 0:bl1]
                )
                emit_tail(prev[0], prev[1], prev[2])
                ytpB = hps_pool.tile([P, c.KD, 1], FP32, tag="h")
                for dblk in range(c.KD):
                    for kc in range(c.KOUT):
                        hd, dc = kc // c.KD, kc % c.KD
                        nc.tensor.matmul(
                            ytpB[:, dblk, :],
                            wout_sb[:, kc, ts(dblk, P)],
                            poolT_sb[:, dc, hd, bl1:bl1 + 1],
                            start=(kc == 0), stop=(kc == c.KOUT - 1),
                        )
                nc.vector.tensor_add(
                    y_sbT[:, :, bl1:bl1 + 1], ytpB[:],
                    boutT_sb[:, :, bl1:bl1 + 1]
                )
                nc.sync.dma_start(y_d[:], y_sbT[:])
    return nc


def plan_slots(valid_mask, n_cores, BL):
    """Count-sort batches into (core, slot) so each SPMD batch-slot has a
    tight shared token bound."""
    counts = np.asarray(valid_mask).sum(1)
    order = np.argsort(counts, kind="stable")[::-1]
    slot_npads = []
    for bl in range(BL):
        grp = order[bl * n_cores:(bl + 1) * n_cores]
        mx = int(counts[grp].max())
        slot_npads.append(max(256, mx))
    return order, slot_npads


def make_in_maps(x, valid_mask, W1, b1, W2, b2, Wout, bout, n_cores, cfg):
    """Host-side prep: w2-sort heads' dims, compact valid tokens, fp8
    layouts, shard over batch."""
    c = cfg
    f8 = ml_dtypes.float8_e4m3
    f16 = np.float16
    B, N, D = x.shape
    H, _, F = W1.shape
    HF = H * F

    def q8(a):
        return np.asarray(a, np.float32).astype(f8)

    # per-head permutation: large |w2| dims first
    perm = [np.argsort(-np.abs(np.asarray(W2[h], np.float32)),
                       kind="stable") for h in range(H)]
    W1p = np.stack([np.asarray(W1[h], np.float32)[:, perm[h]]
                    for h in range(H)])          # [H, D, F]
    b1p = np.stack([np.asarray(b1[h], np.float32)[perm[h]]
                    for h in range(H)])          # [H, F]
    w2p = np.stack([np.asarray(W2[h], np.float32)[perm[h]]
                    for h in range(H)])          # [H, F]

    W1f = W1p.transpose(1, 0, 2).reshape(D, HF)  # [D, HF] head-major cols
    w18 = q8(8.0 * W1f)
    w1r = 8.0 * W1f - w18.astype(np.float32)

    def dr_pack_w(wmat):
        # [D, M] -> [P, D//256, 2, M] with d = dc*256 + i*128 + p
        Dw, M = wmat.shape
        return np.ascontiguousarray(
            wmat.reshape(Dw // 256, 2, P, M).transpose(2, 0, 1, 3)
        )

    w18_l = dr_pack_w(w18)
    # rank-0 columns of each head, head-major compact [D, H*P]
    r0cols = np.concatenate(
        [np.arange(4 * h * P, (4 * h + 1) * P) for h in range(H)]
    )
    w1lo_l = dr_pack_w(q8(w1r[:, r0cols]))

    # score-dot stationaries: w28[p, pc, j, hd] = q8(8*w2[f]) one-hot by
    # head, f = (2*pc+j)*128 + p, head = (2*pc+j)//4.  Head axis padded
    # 4->16 for the DR LdWeights pair-step%16 rule.
    w2f8 = q8(8.0 * w2p.reshape(HF))
    w2flo = q8(8.0 * w2p.reshape(HF) - w2f8.astype(np.float32))
    w28_l = np.zeros((P, c.PC, 2, 16), f8)
    w28lo_l = np.zeros((P, c.PC // 2, 2, 16), f8)
    for pc in range(c.PC):
        for j in range(2):
            mc = 2 * pc + j
            fidx = mc * P + np.arange(P)
            w28_l[np.arange(P), pc, j, mc // 4] = w2f8[fidx]
            if pc % 2 == 0:
                w28lo_l[np.arange(P), pc // 2, j, mc // 4] = w2flo[fidx]
    w28_l = np.ascontiguousarray(w28_l)
    w28lo_l = np.ascontiguousarray(w28lo_l)

    b1T_l = np.ascontiguousarray(
        b1p.reshape(HF).reshape(c.MC, P).transpose(1, 0).astype(np.float32)
    )

    # pad-token score per head at the device's exact precision:
    # h_pad = e4m3(tanh(b1)), s_pad8 = sum_f w28[f]*h_pad[f] (+ w28lo on
    # top pairs); e_pad = exp(s_pad8/8)
    hpadq = np.asarray(np.tanh(b1p.reshape(HF)), np.float32).astype(f8)
    hpadf = hpadq.astype(np.float32)
    w28f = w2f8.astype(np.float32)
    w28lof = w2flo.astype(np.float32)
    s_pad8 = np.zeros(H, np.float64)
    for mc in range(c.MC):
        fidx = mc * P + np.arange(P)
        hd = mc // 4
        s_pad8[hd] += (w28f[fidx] * hpadf[fidx]).sum()
        if (mc // 2) % 2 == 0:   # top pair -> w2lo residual applies
            s_pad8[hd] += (w28lof[fidx] * hpadf[fidx]).sum()
    e_pad = np.exp(s_pad8 / 8.0)                 # [H]

    wout_l = np.ascontiguousarray(
        np.asarray(Wout, np.float32).reshape(c.KOUT, P, c.D)
        .transpose(1, 0, 2).astype(f16)
    )
    boutT_l = np.ascontiguousarray(
        np.broadcast_to(
            np.asarray(bout, np.float32).reshape(c.KD, P)
            .transpose(1, 0)[:, :, None],
            (P, c.KD, c.BL),
        ).astype(np.float32)
    )

    order, slot_npads = plan_slots(valid_mask, n_cores, c.BL)
    for bl in range(c.BL):
        assert slot_npads[bl] <= c.slot_npads[bl], (
            f"slot {bl}: cfg bound {c.slot_npads[bl]} < data {slot_npads[bl]}"
        )
    in_maps = []
    for core in range(n_cores):
        xt8_a = np.zeros((c.BL, P, c.KDR, 2, c.N_pad), f8)
        xlo_a = np.zeros((c.BL, P, c.KDR, 2, c.N_pad), f8)
        xc_a = np.zeros((c.BL, c.N_pad, D), f16)
        zcn_a = np.zeros((c.H, c.BL), np.float32)
        for bl in range(c.BL):
            bg = int(order[bl * n_cores + core])
            idx = np.where(valid_mask[bg])[0]
            cnt = len(idx)
            assert cnt <= c.slot_npads[bl]
            xv = np.asarray(x[bg][idx], np.float32)        # [cnt, D]
            x8 = xv.astype(f8)
            xr = xv - x8.astype(np.float32)
            xlo = xr.astype(f8)
            # [cnt, D] -> [P, D//256, 2, cnt]
            xt = x8.T.reshape(c.KDR, 2, P, cnt).transpose(2, 0, 1, 3)
            xl = xlo.T.reshape(c.KDR, 2, P, cnt).transpose(2, 0, 1, 3)
            xt8_a[bl, :, :, :, :cnt] = xt
            xlo_a[bl, :, :, :, :cnt] = xl
            xc_a[bl, :cnt] = xv.astype(f16)
            zcn_a[:, bl] = -(c.slot_npads[bl] - cnt) * e_pad
        in_maps.append({
            "xt8": np.ascontiguousarray(xt8_a),
            "xlo": np.ascontiguousarray(xlo_a),
            "xc": np.ascontiguousarray(xc_a),
            "w18": w18_l, "w1lo": w1lo_l,
            "w28": w28_l, "w28lo": w28lo_l, "b1T": b1T_l,
            "zcorrn": zcn_a,
            "wout": wout_l, "boutT": boutT_l,
        })
    return in_maps


_cached = {}
last_results = None


def kernel(x, valid_mask, W1, b1, W2, b2, Wout, bout, trace=False):
    global last_results
    x, valid_mask, W1, b1, W2, b2, Wout, bout = (
        np.asarray(a)
        for a in (x, valid_mask, W1, b1, W2, b2, Wout, bout)
    )
    B = x.shape[0]
    n_cores = 8
    BL = B // n_cores
    order, slot_npads = plan_slots(valid_mask, n_cores, BL)
    n_pad = max(slot_npads)
    cfg = Cfg(BL=BL, N_pad=n_pad, slot_npads=slot_npads)
    key = (B, n_pad, tuple(slot_npads))
    if key not in _cached:
        nc = bacc.Bacc("TRN2", target_bir_lowering=False, debug=False)
        build_kernel(nc, cfg)
        nc.compile()
        _cached[key] = nc
    in_maps = make_in_maps(x, valid_mask, W1, b1, W2, b2, Wout, bout,
                           n_cores, cfg)
    res = run_bass_kernel_spmd(
        _cached[key], in_maps, core_ids=list(range(n_cores)), trace=trace
    )
    last_results = res
    y = np.empty((B, cfg.D), np.float32)
    for core in range(n_cores):
        yT = np.asarray(res.results[core]["y"], np.float32)  # [P, KD, BL]
        yc = yT.transpose(2, 1, 0).reshape(BL, cfg.D)
        for bl in range(BL):
            y[int(order[bl * n_cores + core])] = yc[bl]
    return y
